# revision 15
# baseline (speedup 1.0000x reference)
"""CARE-GNN forward on 8 Trainium2 NeuronCores (Bass/Tile).

Strategy (dst-sharded message passing):
  - Nodes are sharded across 8 cores by dst range; each core owns all edges
    into its nodes, so segment sums/counts complete locally (no all-reduce).
  - Per layer, every core reads a full node table [h | pj'] (bf16, 512B rows)
    from HBM, rebuilt via AllGather of per-core slabs.
  - Edges are laid out slot-aligned: the k-th in-edge of the node at tile
    position p lives at partition p of chunk k. Aggregation is then a
    diag(alpha) matmul accumulating into PSUM, and pi[dst] is just the
    per-tile pi column (no per-edge pi gather).
  - att([h_i, h_j]) = h_i @ wi + h_j @ wj, so alpha needs only per-node
    projections: pi' = h @ wi + att_b stays in SBUF, pj = h @ wj rides in
    the gathered row.
  - dma_gather uses signed int16 indices, so the table is addressed in two
    regions split at the core-5 slab boundary (5/8 vs 3/8 of rows, both
    < 32768), one gather call per region per tile; empty slots gather row 0
    and are zeroed by a validity mask folded into alpha.
  - Within a core, nodes are packed into tiles sorted lexicographically by
    (region-A in-degree, region-B in-degree) so the per-tile chunk counts
    (max over the 128 positions) stay tight.
"""

import os
import sys
import types

import numpy as np
import ml_dtypes

N_CORES = 8
HID = 128
ROW_W = 2 * HID  # bf16 elements per table row (512B): [h(128) | pj'(1) | pad]
REG_SPLIT_CORE = 5  # table rows of cores 0..4 are region A, 5..7 region B


def _install_axon_ntff_hook():
    """Best-effort shim so trace=True (BASS_TRACE=1) works under axon."""
    try:
        if "antenv.axon_hooks" in sys.modules:
            return
        mod = types.ModuleType("antenv.axon_hooks")
        mod._hook = None
        mod.set_axon_ntff_profile_hook = lambda h: setattr(mod, "_hook", h)
        mod.get_axon_ntff_profile_hook = lambda: mod._hook
        sys.modules["antenv.axon_hooks"] = mod
        import antenv

        antenv.axon_hooks = mod
        from trn_agent_boot.trn_boot import _ntff_profile_via_ctypes

        so = "/opt/axon/libaxon_pjrt.so"
        if os.path.exists(so):
            mod.set_axon_ntff_profile_hook(_ntff_profile_via_ctypes(so))
    except Exception:
        pass


def _host_prep(x, edge_index):
    """Shard nodes/edges, build slot-aligned gather indices. Pure index work."""
    N = x.shape[0]
    src = np.asarray(edge_index[0], dtype=np.int64)
    dst = np.asarray(edge_index[1], dtype=np.int64)
    npc = (N + N_CORES - 1) // N_CORES  # nodes per core
    tpc = (npc + 127) // 128  # tiles per core
    slab = tpc * 128
    half = REG_SPLIT_CORE * slab  # region-A table rows

    deg = np.bincount(dst, minlength=N)
    owner = np.minimum(np.arange(N) // npc, N_CORES - 1)
    src_reg = (owner[src] >= REG_SPLIT_CORE).astype(np.int64)  # edge region
    lowdeg = np.bincount(dst[src_reg == 0], minlength=N)
    highdeg = deg - lowdeg

    # node -> slab row: tiles packed by (lowdeg, highdeg) desc for tight
    # per-tile chunk counts.
    slabrow = np.empty(N, dtype=np.int64)
    for c in range(N_CORES):
        lo, hi = c * npc, min((c + 1) * npc, N)
        ld, hd = lowdeg[lo:hi], highdeg[lo:hi]
        order = np.lexsort((-hd, -ld))
        # within 4-tile windows, re-sort by highdeg: keeps region-A chunk
        # counts near-tight while pulling region-B counts much tighter
        W = 512
        for s in range(0, hi - lo, W):
            w = order[s : s + W]
            order[s : s + W] = w[np.argsort(-hd[w], kind="stable")]
        slabrow[lo + order] = np.arange(hi - lo)
    table_row = owner * slab + slabrow

    e_src_row = table_row[src]
    e_core = owner[dst]
    e_slabrow = slabrow[dst]
    e_tile = e_slabrow // 128
    e_pos = e_slabrow % 128
    e_reg = src_reg

    # per-(core,tile,pos,region) sequence number -> chunk index
    key = (((e_core * tpc + e_tile) * 128 + e_pos) * 2 + e_reg).astype(np.int64)
    order = np.argsort(key, kind="stable")
    ks = key[order]
    grp_start = np.r_[0, np.flatnonzero(np.diff(ks)) + 1]
    grp_len = np.diff(np.r_[grp_start, len(ks)])
    seq = np.arange(len(ks)) - np.repeat(grp_start, grp_len)
    e_seq = np.empty(len(ks), dtype=np.int64)
    e_seq[order] = seq

    # chunk counts per (core, tile, region) = max over positions of count
    cnt = np.zeros((N_CORES, tpc, 128, 2), dtype=np.int64)
    np.add.at(cnt, (e_core, e_tile, e_pos, e_reg), 1)
    c_reg = cnt.max(axis=2).max(axis=0)  # [tpc, 2] shared across cores (SPMD)
    CA_phys = c_reg[:, 0].astype(int)
    CB_phys = c_reg[:, 1].astype(int)
    assert (CA_phys + CB_phys).min() >= 1

    # processing order: tiles greedily paired into groups of GRP so each
    # group's chunk totals are balanced (gathers are per group+region)
    GRP = 2
    ngrp = (tpc + GRP - 1) // GRP
    torder = np.argsort(-(CA_phys + CB_phys), kind="stable")
    gsum = np.zeros(ngrp)
    gcnt = np.zeros(ngrp, dtype=int)
    groups = [[] for _ in range(ngrp)]
    for t in torder:
        cand = [i for i in range(ngrp) if gcnt[i] < GRP]
        i = min(cand, key=lambda i: gsum[i])
        groups[i].append(int(t))
        gsum[i] += CA_phys[t] + CB_phys[t]
        gcnt[i] += 1
    proc = [t for g in groups for t in g]  # processing slot -> physical tile
    slot_of = np.empty(tpc, dtype=np.int64)
    slot_of[proc] = np.arange(tpc)

    CA = CA_phys[proc]
    CB = CB_phys[proc]
    CT = CA + CB
    offs_a = np.r_[0, np.cumsum(CA)].astype(int)
    offs_b = np.r_[0, np.cumsum(CB)].astype(int)
    offs_t = np.r_[0, np.cumsum(CT)].astype(int)
    tot_a, tot_b, tot_t = int(offs_a[-1]), int(offs_b[-1]), int(offs_t[-1])

    # gather index arrays in PROCESSING order (slot = chunk*128 + pos)
    e_slot = slot_of[e_tile]
    idx_a = np.zeros((N_CORES, max(tot_a, 1) * 128), dtype=np.int64)
    idx_b = np.zeros((N_CORES, max(tot_b, 1) * 128), dtype=np.int64)
    mask = np.zeros((N_CORES, 128, tot_t), dtype=np.float32)

    e_off = np.where(e_reg == 0, offs_a[e_slot] * 128, offs_b[e_slot] * 128)
    e_lin = e_off + e_seq * 128 + e_pos
    e_val = np.where(e_reg == 0, e_src_row, e_src_row - half)
    for c in range(N_CORES):
        m = e_core == c
        ra = m & (e_reg == 0)
        rb = m & (e_reg == 1)
        idx_a[c, e_lin[ra]] = e_val[ra]
        idx_b[c, e_lin[rb]] = e_val[rb]
        # mask columns: slot s occupies [offs_t[s], offs_t[s+1]) = [A.. | B..]
        mask[c, e_pos[ra], offs_t[e_slot[ra]] + e_seq[ra]] = 1.0
        mask[c, e_pos[rb], offs_t[e_slot[rb]] + CA[e_slot[rb]] + e_seq[rb]] = 1.0

    def wrap16(lin):  # [n] -> [128, n//16] int16 (16-part wrap, replicated x8)
        w = lin.reshape(-1, 16).T.astype(np.uint16).view(np.int16)  # [16, n/16]
        return np.tile(w, (8, 1))

    idx_a16 = np.stack([wrap16(idx_a[c]) for c in range(N_CORES)])
    idx_b16 = np.stack([wrap16(idx_b[c]) for c in range(N_CORES)])

    # inverse degree, laid out [pos, tile]; holes -> 1.0
    invdeg = np.ones((N_CORES, slab), dtype=np.float32)
    invdeg[owner, slabrow] = 1.0 / np.maximum(deg, 1).astype(np.float32)
    invdeg = invdeg.reshape(N_CORES, tpc, 128).transpose(0, 2, 1).copy()

    # x slabs, transposed: [in_dim, slab] per core
    in_dim = x.shape[1]
    xT = np.zeros((N_CORES, in_dim, slab), dtype=np.float32)
    for c in range(N_CORES):
        lo, hi = c * npc, min((c + 1) * npc, N)
        xT[c][:, slabrow[lo:hi]] = np.asarray(x[lo:hi], dtype=np.float32).T

    ga_max = max(
        int(offs_a[min(i0 + GRP, tpc)] - offs_a[i0]) for i0 in range(0, tpc, GRP)
    )
    gb_max = max(
        int(offs_b[min(i0 + GRP, tpc)] - offs_b[i0]) for i0 in range(0, tpc, GRP)
    )
    return dict(
        N=N, npc=npc, tpc=tpc, slab=slab, half=half, in_dim=in_dim,
        CA=CA, CB=CB, offs_a=offs_a, offs_b=offs_b, offs_t=offs_t,
        tot_a=tot_a, tot_b=tot_b, tot_t=tot_t,
        idx_a16=idx_a16, idx_b16=idx_b16, mask=mask, invdeg=invdeg, xT=xT,
        owner=owner, slabrow=slabrow,
        proc=proc, GRP=GRP, ga_max=ga_max, gb_max=gb_max,
    )


def _build_program(p, consts):
    import concourse.bacc as bacc
    import concourse.mybir as mybir
    import concourse.tile as tile

    f32 = mybir.dt.float32
    bf16 = mybir.dt.bfloat16
    i16 = mybir.dt.int16
    AF = mybir.ActivationFunctionType

    tpc, slab, in_dim, half = p["tpc"], p["slab"], p["in_dim"], p["half"]
    CA, CB = p["CA"], p["CB"]
    offs_a, offs_b, offs_t = p["offs_a"], p["offs_b"], p["offs_t"]
    tot_a, tot_b, tot_t = p["tot_a"], p["tot_b"], p["tot_t"]
    att_bs = (consts["att1_b"], consts["att2_b"])
    gtab = N_CORES * slab
    nk = in_dim // 128  # contraction tiles for the encoder
    SEG = 16  # msg build granularity in chunks
    proc, GRP = p["proc"], p["GRP"]
    ga_max, gb_max = p["ga_max"], p["gb_max"]

    nc = bacc.Bacc("TRN2", num_devices=N_CORES, num_swdge_queues=4, dynamic_dma_scratch_size=16384)

    # ---- I/O ----
    xT = nc.dram_tensor("xT", [in_dim, slab], f32, kind="ExternalInput")
    idxA = nc.dram_tensor("idxA", [128, max(tot_a, 1) * 8], i16, kind="ExternalInput")
    idxB = nc.dram_tensor("idxB", [128, max(tot_b, 1) * 8], i16, kind="ExternalInput")
    maskT = nc.dram_tensor("maskT", [128, tot_t], bf16, kind="ExternalInput")
    invdeg = nc.dram_tensor("invdeg", [128, tpc], f32, kind="ExternalInput")
    encw = nc.dram_tensor("encw", [in_dim, HID], f32, kind="ExternalInput")
    encb = nc.dram_tensor("encb", [HID, 1], f32, kind="ExternalInput")
    w4 = nc.dram_tensor("w4", [HID, 4], f32, kind="ExternalInput")
    clsw = nc.dram_tensor("clsw", [HID, 2], f32, kind="ExternalInput")
    clsb = nc.dram_tensor("clsb", [1, 2], f32, kind="ExternalInput")
    ident_in = nc.dram_tensor("ident", [128, 128], f32, kind="ExternalInput")
    logits = nc.dram_tensor("logits", [slab, 2], f32, kind="ExternalOutput")

    # ---- internal DRAM ----
    slabs = [nc.dram_tensor(f"slab{l}", [slab, ROW_W], bf16) for l in (1, 2)]
    tables = [
        nc.dram_tensor(f"table{l}", [gtab, ROW_W], bf16, addr_space="Shared")
        for l in (1, 2)
    ]

    with tile.TileContext(nc) as tc:
        with (
            tc.tile_pool(name="const", bufs=1) as cpool,
            tc.tile_pool(name="work", bufs=3) as pool,
            tc.tile_pool(name="dias", bufs=4) as dpool,
            tc.tile_pool(name="gath", bufs=2) as gpool,
            tc.tile_pool(name="psacc", bufs=3, space="PSUM") as ps_acc,
            tc.tile_pool(name="pstr", bufs=3, space="PSUM") as ps_tr,
            tc.tile_pool(name="pssm", bufs=2, space="PSUM") as ps_sm,
        ):
            # ---- constants / whole-kernel residents in SBUF ----
            encw_t = [cpool.tile([128, HID], f32, tag=f"encw{i}", name=f"encw{i}") for i in range(nk)]
            for i, t in enumerate(encw_t):
                nc.sync.dma_start(out=t[:], in_=encw[i * 128 : (i + 1) * 128, :])
            encb_t = cpool.tile([HID, 1], f32, tag="encb")
            nc.sync.dma_start(out=encb_t[:], in_=encb[:])
            w4_t = cpool.tile([HID, 4], f32, tag="w4")
            nc.sync.dma_start(out=w4_t[:], in_=w4[:])
            clsw_t = cpool.tile([HID, 2], f32, tag="clsw")
            nc.sync.dma_start(out=clsw_t[:], in_=clsw[:])
            clsb_t = cpool.tile([1, 2], f32, tag="clsb")
            nc.sync.dma_start(out=clsb_t[:], in_=clsb[:])
            ident_f = cpool.tile([128, 128], f32, tag="identf")
            nc.sync.dma_start(out=ident_f[:], in_=ident_in[:])
            ident_b = cpool.tile([128, 128], bf16, tag="identb")
            nc.vector.tensor_copy(out=ident_b[:], in_=ident_f[:])
            ones_f = cpool.tile([1, 128], f32, tag="onesf")
            nc.vector.memset(ones_f[:], 1.0)
            inv_all = cpool.tile([128, tpc], f32, tag="invall")
            nc.sync.dma_start(out=inv_all[:], in_=invdeg[:])
            idxA_t = cpool.tile([128, max(tot_a, 1) * 8], i16, tag="idxAt")
            nc.sync.dma_start(out=idxA_t[:], in_=idxA[:])
            idxB_t = cpool.tile([128, max(tot_b, 1) * 8], i16, tag="idxBt")
            nc.sync.dma_start(out=idxB_t[:], in_=idxB[:])
            mask_all = cpool.tile([128, tot_t], bf16, tag="maskall")
            nc.sync.dma_start(out=mask_all[:], in_=maskT[:])
            pi_all = [
                cpool.tile([128, tpc], f32, tag=f"piall{l}", name=f"piall{l}")
                for l in (1, 2)
            ]

            def p_phase_and_store(hT_sb, t, layer):
                """hT (f32 [hid, nodes]) -> slab rows [h|pj'] + pi' column."""
                co = t * 128
                lw = 2 * (layer - 1)
                h_ps = ps_tr.tile([128, 128], f32, tag="tr")
                nc.tensor.transpose(out=h_ps[:], in_=hT_sb[:], identity=ident_f[:])
                h_sb = pool.tile([128, 128], bf16, tag="hsb")
                nc.vector.tensor_copy(out=h_sb[:], in_=h_ps[:])
                nc.sync.dma_start(
                    out=slabs[layer - 1][co : co + 128, 0:HID], in_=h_sb[:]
                )
                p_ps = ps_sm.tile([128, 2], f32, tag="sm")
                nc.tensor.matmul(
                    out=p_ps[:], lhsT=hT_sb[:], rhs=w4_t[:, lw : lw + 2],
                    start=True, stop=True,
                )
                # pi' = pi + att_b kept in SBUF for the sigmoid bias
                nc.scalar.add(
                    out=pi_all[layer - 1][:, t : t + 1],
                    in_=p_ps[:, 0:1],
                    add=float(att_bs[layer - 1]),
                )
                pj_ext = pool.tile([128, ROW_W - HID], bf16, tag="pjext")
                nc.vector.memset(pj_ext[:], 0.0)
                nc.vector.tensor_copy(out=pj_ext[:, 0:1], in_=p_ps[:, 1:2])
                nc.sync.dma_start(
                    out=slabs[layer - 1][co : co + 128, HID:ROW_W], in_=pj_ext[:]
                )

            # ---- encoder: hT = relu(encw.T @ xT + encb), then p1 ----
            for t in range(tpc):
                co = t * 128
                xt = [pool.tile([128, 128], f32, tag=f"xt{i}", name=f"xt{i}") for i in range(nk)]
                for i, xx in enumerate(xt):
                    nc.sync.dma_start(
                        out=xx[:], in_=xT[i * 128 : (i + 1) * 128, co : co + 128]
                    )
                hT_ps = ps_tr.tile([128, 128], f32, tag="tr")
                for i in range(nk):
                    nc.tensor.matmul(
                        out=hT_ps[:], lhsT=encw_t[i][:], rhs=xt[i][:],
                        start=(i == 0), stop=(i == nk - 1),
                    )
                hT_sb = pool.tile([128, 128], f32, tag="hTsb")
                nc.scalar.activation(
                    out=hT_sb[:], in_=hT_ps[:], func=AF.Relu, bias=encb_t[:]
                )
                p_phase_and_store(hT_sb, t, layer=1)

            # ---- two message-passing layers ----
            gather_ctr = [0]
            for layer in (1, 2):
                table = tables[layer - 1]
                nc.gpsimd.collective_compute(
                    "AllGather",
                    mybir.AluOpType.bypass,
                    replica_groups=[list(range(N_CORES))],
                    ins=[slabs[layer - 1][:]],
                    outs=[table[:]],
                )
                for i0 in range(0, tpc, GRP):
                    i1 = min(i0 + GRP, tpc)
                    # one gather per region covering tiles i0..i1-1
                    g = []
                    for reg, idx_t, offs, base, rows, gmax in (
                        (0, idxA_t, offs_a, 0, min(half, gtab), ga_max),
                        (1, idxB_t, offs_b, half, max(gtab - half, 0), gb_max),
                    ):
                        cw = int(offs[i1] - offs[i0])
                        gt = gpool.tile(
                            [128, gmax, ROW_W], bf16, tag=f"g{reg}", name=f"g{reg}"
                        )
                        nc.gpsimd.dma_gather(
                            out_ap=gt[:, 0:cw, :],
                            in_ap=table[base : base + rows, :],
                            idxs_ap=idx_t[:, offs[i0] * 8 : offs[i1] * 8],
                            num_idxs=cw * 128,
                            num_idxs_reg=cw * 128,
                            elem_size=ROW_W,
                            single_packet=False,
                            queue_num=gather_ctr[0] % 4,
                        )
                        gather_ctr[0] += 1
                        g.append(gt)
                    for i in range(i0, i1):
                        t = proc[i]
                        co = t * 128
                        ca, cb = int(CA[i]), int(CB[i])
                        ct = ca + cb
                        aoff_a = int(offs_a[i] - offs_a[i0])
                        aoff_b = int(offs_b[i] - offs_b[i0])
                        pi_col = pi_all[layer - 1][:, t : t + 1]
                        # alpha = sigmoid(pj + pi') * mask (bf16 [128, ct])
                        alpha = pool.tile([128, ct], bf16, tag="alpha")
                        if ca:
                            nc.scalar.activation(
                                out=alpha[:, 0:ca, None],
                                in_=g[0][:, aoff_a : aoff_a + ca, HID : HID + 1],
                                func=AF.Sigmoid,
                                bias=pi_col,
                            )
                        if cb:
                            nc.scalar.activation(
                                out=alpha[:, ca:ct, None],
                                in_=g[1][:, aoff_b : aoff_b + cb, HID : HID + 1],
                                func=AF.Sigmoid,
                                bias=pi_col,
                            )
                        nc.vector.tensor_tensor(
                            out=alpha[:],
                            in0=alpha[:],
                            in1=mask_all[:, offs_t[i] : offs_t[i] + ct],
                            op=mybir.AluOpType.mult,
                        )
                        # msg = alpha (*) rows in groups of SEG; slot-sum via
                        # identity-matmul PSUM accumulation
                        acc = ps_acc.tile([128, HID], f32, tag="acc")
                        for g0 in range(0, ct, SEG):
                            gw = min(SEG, ct - g0)
                            msg = dpool.tile([128, SEG, 128], bf16, tag="msg")
                            for lo, hi, reg, aoff in (
                                (g0, min(ca, g0 + gw), 0, aoff_a),
                                (max(ca, g0), g0 + gw, 1, aoff_b - ca),
                            ):
                                if hi <= lo:
                                    continue
                                nc.vector.tensor_tensor(
                                    out=msg[:, lo - g0 : hi - g0, :],
                                    in0=g[reg][:, aoff + lo : aoff + hi, 0:HID],
                                    in1=alpha[:, lo:hi, None].to_broadcast(
                                        [128, hi - lo, 128]
                                    ),
                                    op=mybir.AluOpType.mult,
                                )
                            for k in range(g0, g0 + gw):
                                nc.tensor.matmul(
                                    out=acc[:],
                                    lhsT=ident_b[:],
                                    rhs=msg[:, k - g0, :],
                                    start=(k == 0),
                                    stop=(k == ct - 1),
                                )
                        inv_col = inv_all[:, t : t + 1]
                        if layer == 1:
                            # h2 = relu(acc * invdeg); p2 phase + stores
                            h2_sb = pool.tile([128, 128], f32, tag="h2sb")
                            nc.scalar.activation(
                                out=h2_sb[:], in_=acc[:], func=AF.Relu, scale=inv_col
                            )
                            hT2_ps = ps_tr.tile([128, 128], f32, tag="tr")
                            nc.tensor.transpose(
                                out=hT2_ps[:], in_=h2_sb[:], identity=ident_f[:]
                            )
                            hT2_sb = pool.tile([128, 128], f32, tag="hT2sb")
                            nc.vector.tensor_copy(out=hT2_sb[:], in_=hT2_ps[:])
                            p_phase_and_store(hT2_sb, t, layer=2)
                        else:
                            # logits = (acc * invdeg) @ clsw + clsb
                            m_sb = pool.tile([128, 128], f32, tag="msb")
                            nc.scalar.mul(out=m_sb[:], in_=acc[:], mul=inv_col)
                            mT_ps = ps_tr.tile([128, 128], f32, tag="tr")
                            nc.tensor.transpose(
                                out=mT_ps[:], in_=m_sb[:], identity=ident_f[:]
                            )
                            mT_sb = pool.tile([128, 128], f32, tag="mTsb")
                            nc.vector.tensor_copy(out=mT_sb[:], in_=mT_ps[:])
                            lg_ps = ps_sm.tile([128, 2], f32, tag="sm")
                            nc.tensor.matmul(
                                out=lg_ps[:], lhsT=mT_sb[:], rhs=clsw_t[:],
                                start=True, stop=False,
                            )
                            nc.tensor.matmul(
                                out=lg_ps[:], lhsT=ones_f[:], rhs=clsb_t[:],
                                start=False, stop=True,
                            )
                            lg_sb = pool.tile([128, 2], f32, tag="lgsb")
                            nc.vector.tensor_copy(out=lg_sb[:], in_=lg_ps[:])
                            nc.sync.dma_start(
                                out=logits[co : co + 128, :], in_=lg_sb[:]
                            )

    nc.compile()
    return nc


_CACHE = {}


def kernel(**inputs):
    _install_axon_ntff_hook()
    from concourse import bass_utils

    bass_utils.upload_artifacts = lambda tmpdir: tmpdir

    x = np.asarray(inputs["x"], dtype=np.float32)
    edge_index = np.asarray(inputs["edge_index"])
    p = _host_prep(x, edge_index)

    consts = dict(
        att1_b=float(np.asarray(inputs["att1_b"]).reshape(-1)[0]),
        att2_b=float(np.asarray(inputs["att2_b"]).reshape(-1)[0]),
    )
    key = (tuple(p["CA"]), tuple(p["CB"]), consts["att1_b"], consts["att2_b"])
    if key not in _CACHE:
        _CACHE[key] = _build_program(p, consts)
    nc = _CACHE[key]

    w4 = np.concatenate(
        [
            np.asarray(inputs["att1_w"], dtype=np.float32).reshape(2, HID).T,
            np.asarray(inputs["att2_w"], dtype=np.float32).reshape(2, HID).T,
        ],
        axis=1,
    )  # [HID, 4] = [wi1, wj1, wi2, wj2]
    common = dict(
        encw=np.ascontiguousarray(np.asarray(inputs["enc_w"], dtype=np.float32)),
        encb=np.asarray(inputs["enc_b"], dtype=np.float32).reshape(HID, 1),
        w4=np.ascontiguousarray(w4),
        clsw=np.ascontiguousarray(np.asarray(inputs["cls_w"], dtype=np.float32)),
        clsb=np.asarray(inputs["cls_b"], dtype=np.float32).reshape(1, 2),
        ident=np.eye(128, dtype=np.float32),
    )
    in_maps = []
    for c in range(N_CORES):
        in_maps.append(
            dict(
                xT=np.ascontiguousarray(p["xT"][c]),
                idxA=np.ascontiguousarray(p["idx_a16"][c]),
                idxB=np.ascontiguousarray(p["idx_b16"][c]),
                maskT=np.ascontiguousarray(p["mask"][c].astype(ml_dtypes.bfloat16)),
                invdeg=np.ascontiguousarray(p["invdeg"][c]),
                **common,
            )
        )

    res = bass_utils.run_bass_kernel_spmd(nc, in_maps, core_ids=list(range(N_CORES)))
    kernel.last_result = res

    N = p["N"]
    out = np.zeros((N, 2), dtype=np.float32)
    for c in range(N_CORES):
        m = p["owner"] == c
        out[m] = np.asarray(res.results[c]["logits"], dtype=np.float32)[
            p["slabrow"][m]
        ]
    return out



# revision 22
# speedup vs baseline: 1.1784x; 1.1784x over previous
"""CARE-GNN forward on 8 Trainium2 NeuronCores (Bass/Tile).

Strategy (dst-sharded message passing):
  - Nodes are sharded across 8 cores by dst range; each core owns all edges
    into its nodes, so segment sums/counts complete locally (no all-reduce).
  - Per layer, every core reads a full node table [h | pj'] (bf16, 512B rows)
    from HBM, rebuilt via AllGather of per-core slabs.
  - Edges are laid out slot-aligned: the k-th in-edge of the node at tile
    position p lives at partition p of chunk k. Aggregation is then a
    diag(alpha) matmul accumulating into PSUM, and pi[dst] is just the
    per-tile pi column (no per-edge pi gather).
  - att([h_i, h_j]) = h_i @ wi + h_j @ wj, so alpha needs only per-node
    projections: pi' = h @ wi + att_b stays in SBUF, pj = h @ wj rides in
    the gathered row.
  - dma_gather uses signed int16 indices, so the table is addressed in two
    regions split at the core-5 slab boundary (5/8 vs 3/8 of rows, both
    < 32768), one gather call per region per tile; empty slots gather row 0
    and are zeroed by a validity mask folded into alpha.
  - Within a core, nodes are packed into tiles sorted lexicographically by
    (region-A in-degree, region-B in-degree) so the per-tile chunk counts
    (max over the 128 positions) stay tight.
"""

import os
import sys
import types

import numpy as np
import ml_dtypes

N_CORES = 8
HID = 128
ROW_W = 2 * HID  # bf16 elements per table row (512B): [h(128) | pj'(1) | pad]
REG_SPLIT_CORE = 5  # table rows of cores 0..4 are region A, 5..7 region B


def _install_axon_ntff_hook():
    """Best-effort shim so trace=True (BASS_TRACE=1) works under axon."""
    try:
        if "antenv.axon_hooks" in sys.modules:
            return
        mod = types.ModuleType("antenv.axon_hooks")
        mod._hook = None
        mod.set_axon_ntff_profile_hook = lambda h: setattr(mod, "_hook", h)
        mod.get_axon_ntff_profile_hook = lambda: mod._hook
        sys.modules["antenv.axon_hooks"] = mod
        import antenv

        antenv.axon_hooks = mod
        from trn_agent_boot.trn_boot import _ntff_profile_via_ctypes

        so = "/opt/axon/libaxon_pjrt.so"
        if os.path.exists(so):
            mod.set_axon_ntff_profile_hook(_ntff_profile_via_ctypes(so))
    except Exception:
        pass


def _host_prep(x, edge_index):
    """Shard nodes/edges, build slot-aligned gather indices. Pure index work."""
    N = x.shape[0]
    src = np.asarray(edge_index[0], dtype=np.int64)
    dst = np.asarray(edge_index[1], dtype=np.int64)
    npc = (N + N_CORES - 1) // N_CORES  # nodes per core
    tpc = (npc + 127) // 128  # tiles per core
    slab = tpc * 128
    half = REG_SPLIT_CORE * slab  # region-A table rows

    deg = np.bincount(dst, minlength=N)
    owner = np.minimum(np.arange(N) // npc, N_CORES - 1)
    src_reg = (owner[src] >= REG_SPLIT_CORE).astype(np.int64)  # edge region
    lowdeg = np.bincount(dst[src_reg == 0], minlength=N)
    highdeg = deg - lowdeg

    # node -> slab row: tiles packed by (lowdeg, highdeg) desc for tight
    # per-tile chunk counts.
    slabrow = np.empty(N, dtype=np.int64)
    for c in range(N_CORES):
        lo, hi = c * npc, min((c + 1) * npc, N)
        ld, hd = lowdeg[lo:hi], highdeg[lo:hi]
        order = np.lexsort((-hd, -ld))
        # within 4-tile windows, re-sort by highdeg: keeps region-A chunk
        # counts near-tight while pulling region-B counts much tighter
        W = 512
        for s in range(0, hi - lo, W):
            w = order[s : s + W]
            order[s : s + W] = w[np.argsort(-hd[w], kind="stable")]
        slabrow[lo + order] = np.arange(hi - lo)
    table_row = owner * slab + slabrow

    e_src_row = table_row[src]
    e_core = owner[dst]
    e_slabrow = slabrow[dst]
    e_tile = e_slabrow // 128
    e_pos = e_slabrow % 128
    e_reg = src_reg

    # per-(core,tile,pos,region) sequence number -> chunk index
    key = (((e_core * tpc + e_tile) * 128 + e_pos) * 2 + e_reg).astype(np.int64)
    order = np.argsort(key, kind="stable")
    ks = key[order]
    grp_start = np.r_[0, np.flatnonzero(np.diff(ks)) + 1]
    grp_len = np.diff(np.r_[grp_start, len(ks)])
    seq = np.arange(len(ks)) - np.repeat(grp_start, grp_len)
    e_seq = np.empty(len(ks), dtype=np.int64)
    e_seq[order] = seq

    # chunk counts per (core, tile, region) = max over positions of count
    cnt = np.zeros((N_CORES, tpc, 128, 2), dtype=np.int64)
    np.add.at(cnt, (e_core, e_tile, e_pos, e_reg), 1)
    c_reg = cnt.max(axis=2).max(axis=0)  # [tpc, 2] shared across cores (SPMD)
    CA_phys = c_reg[:, 0].astype(int)
    CB_phys = c_reg[:, 1].astype(int)
    assert (CA_phys + CB_phys).min() >= 1

    # processing order: tiles greedily paired into groups of GRP so each
    # group's chunk totals are balanced (gathers are per group+region)
    GRP = 2
    ngrp = (tpc + GRP - 1) // GRP
    torder = np.argsort(-(CA_phys + CB_phys), kind="stable")
    gsum = np.zeros(ngrp)
    gcnt = np.zeros(ngrp, dtype=int)
    groups = [[] for _ in range(ngrp)]
    for t in torder:
        cand = [i for i in range(ngrp) if gcnt[i] < GRP]
        i = min(cand, key=lambda i: gsum[i])
        groups[i].append(int(t))
        gsum[i] += CA_phys[t] + CB_phys[t]
        gcnt[i] += 1
    proc = [t for g in groups for t in g]  # processing slot -> physical tile
    slot_of = np.empty(tpc, dtype=np.int64)
    slot_of[proc] = np.arange(tpc)

    CA = CA_phys[proc]
    CB = CB_phys[proc]
    CT = CA + CB
    offs_a = np.r_[0, np.cumsum(CA)].astype(int)
    offs_b = np.r_[0, np.cumsum(CB)].astype(int)
    offs_t = np.r_[0, np.cumsum(CT)].astype(int)
    tot_a, tot_b, tot_t = int(offs_a[-1]), int(offs_b[-1]), int(offs_t[-1])

    # gather index arrays in PROCESSING order (slot = chunk*128 + pos)
    e_slot = slot_of[e_tile]
    idx_a = np.zeros((N_CORES, max(tot_a, 1) * 128), dtype=np.int64)
    idx_b = np.zeros((N_CORES, max(tot_b, 1) * 128), dtype=np.int64)
    mask = np.zeros((N_CORES, 128, tot_t), dtype=np.float32)

    e_off = np.where(e_reg == 0, offs_a[e_slot] * 128, offs_b[e_slot] * 128)
    e_lin = e_off + e_seq * 128 + e_pos
    e_val = np.where(e_reg == 0, e_src_row, e_src_row - half)
    for c in range(N_CORES):
        m = e_core == c
        ra = m & (e_reg == 0)
        rb = m & (e_reg == 1)
        idx_a[c, e_lin[ra]] = e_val[ra]
        idx_b[c, e_lin[rb]] = e_val[rb]
        # mask columns: slot s occupies [offs_t[s], offs_t[s+1]) = [A.. | B..]
        mask[c, e_pos[ra], offs_t[e_slot[ra]] + e_seq[ra]] = 1.0
        mask[c, e_pos[rb], offs_t[e_slot[rb]] + CA[e_slot[rb]] + e_seq[rb]] = 1.0

    def wrap16(lin):  # [n] -> [128, n//16] int16 (16-part wrap, replicated x8)
        w = lin.reshape(-1, 16).T.astype(np.uint16).view(np.int16)  # [16, n/16]
        return np.tile(w, (8, 1))

    idx_a16 = np.stack([wrap16(idx_a[c]) for c in range(N_CORES)])
    idx_b16 = np.stack([wrap16(idx_b[c]) for c in range(N_CORES)])

    # inverse degree, laid out [pos, tile]; holes -> 1.0
    invdeg = np.ones((N_CORES, slab), dtype=np.float32)
    invdeg[owner, slabrow] = 1.0 / np.maximum(deg, 1).astype(np.float32)
    invdeg = invdeg.reshape(N_CORES, tpc, 128).transpose(0, 2, 1).copy()

    # x slabs, transposed: [in_dim, slab] per core
    in_dim = x.shape[1]
    xT = np.zeros((N_CORES, in_dim, slab), dtype=np.float32)
    for c in range(N_CORES):
        lo, hi = c * npc, min((c + 1) * npc, N)
        xT[c][:, slabrow[lo:hi]] = np.asarray(x[lo:hi], dtype=np.float32).T

    ga_max = max(
        int(offs_a[min(i0 + GRP, tpc)] - offs_a[i0]) for i0 in range(0, tpc, GRP)
    )
    gb_max = max(
        int(offs_b[min(i0 + GRP, tpc)] - offs_b[i0]) for i0 in range(0, tpc, GRP)
    )
    return dict(
        N=N, npc=npc, tpc=tpc, slab=slab, half=half, in_dim=in_dim,
        CA=CA, CB=CB, offs_a=offs_a, offs_b=offs_b, offs_t=offs_t,
        tot_a=tot_a, tot_b=tot_b, tot_t=tot_t,
        idx_a16=idx_a16, idx_b16=idx_b16, mask=mask, invdeg=invdeg, xT=xT,
        owner=owner, slabrow=slabrow,
        proc=proc, GRP=GRP, ga_max=ga_max, gb_max=gb_max,
    )


def _build_program(p, consts):
    import concourse.bacc as bacc
    import concourse.mybir as mybir
    import concourse.tile as tile

    f32 = mybir.dt.float32
    bf16 = mybir.dt.bfloat16
    i16 = mybir.dt.int16
    AF = mybir.ActivationFunctionType

    tpc, slab, in_dim, half = p["tpc"], p["slab"], p["in_dim"], p["half"]
    CA, CB = p["CA"], p["CB"]
    offs_a, offs_b, offs_t = p["offs_a"], p["offs_b"], p["offs_t"]
    tot_a, tot_b, tot_t = p["tot_a"], p["tot_b"], p["tot_t"]
    att_bs = (consts["att1_b"], consts["att2_b"])
    gtab = N_CORES * slab
    nk = in_dim // 128  # contraction tiles for the encoder
    SEG = 16  # msg build granularity in chunks
    proc, GRP = p["proc"], p["GRP"]
    ga_max, gb_max = p["ga_max"], p["gb_max"]

    nc = bacc.Bacc("TRN2", num_devices=N_CORES, num_swdge_queues=4, dynamic_dma_scratch_size=49152)

    # ---- I/O ----
    xT = nc.dram_tensor("xT", [in_dim, slab], f32, kind="ExternalInput")
    idxA = nc.dram_tensor("idxA", [128, max(tot_a, 1) * 8], i16, kind="ExternalInput")
    idxB = nc.dram_tensor("idxB", [128, max(tot_b, 1) * 8], i16, kind="ExternalInput")
    maskT = nc.dram_tensor("maskT", [128, tot_t], bf16, kind="ExternalInput")
    invdeg = nc.dram_tensor("invdeg", [128, tpc], f32, kind="ExternalInput")
    encw = nc.dram_tensor("encw", [in_dim, HID], f32, kind="ExternalInput")
    encb = nc.dram_tensor("encb", [HID, 1], f32, kind="ExternalInput")
    w4 = nc.dram_tensor("w4", [HID, 4], f32, kind="ExternalInput")
    clsw = nc.dram_tensor("clsw", [HID, 2], f32, kind="ExternalInput")
    clsb = nc.dram_tensor("clsb", [1, 2], f32, kind="ExternalInput")
    ident_in = nc.dram_tensor("ident", [128, 128], f32, kind="ExternalInput")
    logits = nc.dram_tensor("logits", [slab, 2], f32, kind="ExternalOutput")

    # ---- internal DRAM ----
    slabs = [nc.dram_tensor(f"slab{l}", [slab, ROW_W], bf16) for l in (1, 2)]
    tables = [
        nc.dram_tensor(f"table{l}", [gtab, ROW_W], bf16, addr_space="Shared")
        for l in (1, 2)
    ]

    with tile.TileContext(nc) as tc:
        with (
            tc.tile_pool(name="const", bufs=1) as cpool,
            tc.tile_pool(name="work", bufs=3) as pool,
            tc.tile_pool(name="dias", bufs=4) as dpool,
            tc.tile_pool(name="gath", bufs=8) as gpool,
            tc.tile_pool(name="psacc", bufs=3, space="PSUM") as ps_acc,
            tc.tile_pool(name="pstr", bufs=3, space="PSUM") as ps_tr,
            tc.tile_pool(name="pssm", bufs=2, space="PSUM") as ps_sm,
        ):
            # ---- constants / whole-kernel residents in SBUF ----
            encw_t = [cpool.tile([128, HID], f32, tag=f"encw{i}", name=f"encw{i}") for i in range(nk)]
            for i, t in enumerate(encw_t):
                nc.sync.dma_start(out=t[:], in_=encw[i * 128 : (i + 1) * 128, :])
            encb_t = cpool.tile([HID, 1], f32, tag="encb")
            nc.sync.dma_start(out=encb_t[:], in_=encb[:])
            w4_t = cpool.tile([HID, 4], f32, tag="w4")
            nc.sync.dma_start(out=w4_t[:], in_=w4[:])
            clsw_t = cpool.tile([HID, 2], f32, tag="clsw")
            nc.sync.dma_start(out=clsw_t[:], in_=clsw[:])
            clsb_t = cpool.tile([1, 2], f32, tag="clsb")
            nc.sync.dma_start(out=clsb_t[:], in_=clsb[:])
            ident_f = cpool.tile([128, 128], f32, tag="identf")
            nc.sync.dma_start(out=ident_f[:], in_=ident_in[:])
            ident_b = cpool.tile([128, 128], bf16, tag="identb")
            nc.vector.tensor_copy(out=ident_b[:], in_=ident_f[:])
            ones_f = cpool.tile([1, 128], f32, tag="onesf")
            nc.vector.memset(ones_f[:], 1.0)
            inv_all = cpool.tile([128, tpc], f32, tag="invall")
            nc.sync.dma_start(out=inv_all[:], in_=invdeg[:])
            idxA_t = cpool.tile([128, max(tot_a, 1) * 8], i16, tag="idxAt")
            nc.sync.dma_start(out=idxA_t[:], in_=idxA[:])
            idxB_t = cpool.tile([128, max(tot_b, 1) * 8], i16, tag="idxBt")
            nc.sync.dma_start(out=idxB_t[:], in_=idxB[:])
            mask_all = cpool.tile([128, tot_t], bf16, tag="maskall")
            nc.sync.dma_start(out=mask_all[:], in_=maskT[:])
            pi_all = [
                cpool.tile([128, tpc], f32, tag=f"piall{l}", name=f"piall{l}")
                for l in (1, 2)
            ]

            def p_phase_and_store(hT_sb, t, layer):
                """hT (f32 [hid, nodes]) -> slab rows [h|pj'] + pi' column."""
                co = t * 128
                lw = 2 * (layer - 1)
                h_ps = ps_tr.tile([128, 128], f32, tag="tr")
                nc.tensor.transpose(out=h_ps[:], in_=hT_sb[:], identity=ident_f[:])
                h_sb = pool.tile([128, 128], bf16, tag="hsb")
                nc.vector.tensor_copy(out=h_sb[:], in_=h_ps[:])
                nc.sync.dma_start(
                    out=slabs[layer - 1][co : co + 128, 0:HID], in_=h_sb[:]
                )
                p_ps = ps_sm.tile([128, 2], f32, tag="sm")
                nc.tensor.matmul(
                    out=p_ps[:], lhsT=hT_sb[:], rhs=w4_t[:, lw : lw + 2],
                    start=True, stop=True,
                )
                # pi' = pi + att_b kept in SBUF for the sigmoid bias
                nc.scalar.add(
                    out=pi_all[layer - 1][:, t : t + 1],
                    in_=p_ps[:, 0:1],
                    add=float(att_bs[layer - 1]),
                )
                pj_ext = pool.tile([128, ROW_W - HID], bf16, tag="pjext")
                nc.vector.memset(pj_ext[:], 0.0)
                nc.vector.tensor_copy(out=pj_ext[:, 0:1], in_=p_ps[:, 1:2])
                nc.sync.dma_start(
                    out=slabs[layer - 1][co : co + 128, HID:ROW_W], in_=pj_ext[:]
                )

            # ---- encoder: hT = relu(encw.T @ xT + encb), then p1 ----
            for t in range(tpc):
                co = t * 128
                xt = [pool.tile([128, 128], f32, tag=f"xt{i}", name=f"xt{i}") for i in range(nk)]
                for i, xx in enumerate(xt):
                    nc.sync.dma_start(
                        out=xx[:], in_=xT[i * 128 : (i + 1) * 128, co : co + 128]
                    )
                hT_ps = ps_tr.tile([128, 128], f32, tag="tr")
                for i in range(nk):
                    nc.tensor.matmul(
                        out=hT_ps[:], lhsT=encw_t[i][:], rhs=xt[i][:],
                        start=(i == 0), stop=(i == nk - 1),
                    )
                hT_sb = pool.tile([128, 128], f32, tag="hTsb")
                nc.scalar.activation(
                    out=hT_sb[:], in_=hT_ps[:], func=AF.Relu, bias=encb_t[:]
                )
                p_phase_and_store(hT_sb, t, layer=1)

            # ---- two message-passing layers ----
            gather_ctr = [0]
            for layer in (1, 2):
                table = tables[layer - 1]
                nc.gpsimd.collective_compute(
                    "AllGather",
                    mybir.AluOpType.bypass,
                    replica_groups=[list(range(N_CORES))],
                    ins=[slabs[layer - 1][:]],
                    outs=[table[:]],
                )
                for i in range(tpc):
                    t = proc[i]
                    co = t * 128
                    ca, cb = int(CA[i]), int(CB[i])
                    ct = ca + cb
                    # uniform gather segments of <= SEG chunks, each segment
                    # gathered / sigmoided / weighted / accumulated on its own
                    # so its buffer recycles immediately
                    segs = []
                    for c0 in range(0, ca, SEG):
                        segs.append((0, c0, min(SEG, ca - c0), c0))
                    for c0 in range(0, cb, SEG):
                        segs.append((1, c0, min(SEG, cb - c0), ca + c0))
                    alpha = pool.tile([128, ct], bf16, tag="alpha")
                    acc = ps_acc.tile([128, HID], f32, tag="acc")
                    pi_col = pi_all[layer - 1][:, t : t + 1]
                    for si, (reg, c0, cw, aoff) in enumerate(segs):
                        idx_t, offs, base, rows = (
                            (idxA_t, offs_a, 0, min(half, gtab))
                            if reg == 0
                            else (idxB_t, offs_b, half, max(gtab - half, 0))
                        )
                        gt = gpool.tile([128, SEG, ROW_W], bf16, tag="g", name="g")
                        nc.gpsimd.dma_gather(
                            out_ap=gt[:, 0:cw, :],
                            in_ap=table[base : base + rows, :],
                            idxs_ap=idx_t[
                                :, (offs[i] + c0) * 8 : (offs[i] + c0 + cw) * 8
                            ],
                            num_idxs=cw * 128,
                            num_idxs_reg=cw * 128,
                            elem_size=ROW_W,
                            single_packet=False,
                            queue_num=gather_ctr[0] % 4,
                        )
                        gather_ctr[0] += 1
                        # alpha = sigmoid(pj + pi') * mask for this segment
                        nc.scalar.activation(
                            out=alpha[:, aoff : aoff + cw, None],
                            in_=gt[:, 0:cw, HID : HID + 1],
                            func=AF.Sigmoid,
                            bias=pi_col,
                        )
                        nc.vector.tensor_tensor(
                            out=alpha[:, aoff : aoff + cw],
                            in0=alpha[:, aoff : aoff + cw],
                            in1=mask_all[
                                :, offs_t[i] + aoff : offs_t[i] + aoff + cw
                            ],
                            op=mybir.AluOpType.mult,
                        )
                        # msg = alpha (*) rows; slot-sum via identity-matmul
                        # PSUM accumulation (acc[pos] += msg[pos, k, :])
                        msg = dpool.tile([128, SEG, 128], bf16, tag="msg")
                        nc.vector.tensor_tensor(
                            out=msg[:, 0:cw, :],
                            in0=gt[:, 0:cw, 0:HID],
                            in1=alpha[:, aoff : aoff + cw, None].to_broadcast(
                                [128, cw, 128]
                            ),
                            op=mybir.AluOpType.mult,
                        )
                        for k in range(cw):
                            nc.tensor.matmul(
                                out=acc[:],
                                lhsT=ident_b[:],
                                rhs=msg[:, k, :],
                                start=(si == 0 and k == 0),
                                stop=(si == len(segs) - 1 and k == cw - 1),
                            )
                    inv_col = inv_all[:, t : t + 1]
                        if layer == 1:
                            # h2 = relu(acc * invdeg); p2 phase + stores
                            h2_sb = pool.tile([128, 128], f32, tag="h2sb")
                            nc.scalar.activation(
                                out=h2_sb[:], in_=acc[:], func=AF.Relu, scale=inv_col
                            )
                            hT2_ps = ps_tr.tile([128, 128], f32, tag="tr")
                            nc.tensor.transpose(
                                out=hT2_ps[:], in_=h2_sb[:], identity=ident_f[:]
                            )
                            hT2_sb = pool.tile([128, 128], f32, tag="hT2sb")
                            nc.vector.tensor_copy(out=hT2_sb[:], in_=hT2_ps[:])
                            p_phase_and_store(hT2_sb, t, layer=2)
                        else:
                            # logits = (acc * invdeg) @ clsw + clsb
                            m_sb = pool.tile([128, 128], f32, tag="msb")
                            nc.scalar.mul(out=m_sb[:], in_=acc[:], mul=inv_col)
                            mT_ps = ps_tr.tile([128, 128], f32, tag="tr")
                            nc.tensor.transpose(
                                out=mT_ps[:], in_=m_sb[:], identity=ident_f[:]
                            )
                            mT_sb = pool.tile([128, 128], f32, tag="mTsb")
                            nc.vector.tensor_copy(out=mT_sb[:], in_=mT_ps[:])
                            lg_ps = ps_sm.tile([128, 2], f32, tag="sm")
                            nc.tensor.matmul(
                                out=lg_ps[:], lhsT=mT_sb[:], rhs=clsw_t[:],
                                start=True, stop=False,
                            )
                            nc.tensor.matmul(
                                out=lg_ps[:], lhsT=ones_f[:], rhs=clsb_t[:],
                                start=False, stop=True,
                            )
                            lg_sb = pool.tile([128, 2], f32, tag="lgsb")
                            nc.vector.tensor_copy(out=lg_sb[:], in_=lg_ps[:])
                            nc.sync.dma_start(
                                out=logits[co : co + 128, :], in_=lg_sb[:]
                            )

    nc.compile()
    return nc


_CACHE = {}


def kernel(**inputs):
    _install_axon_ntff_hook()
    from concourse import bass_utils

    bass_utils.upload_artifacts = lambda tmpdir: tmpdir

    x = np.asarray(inputs["x"], dtype=np.float32)
    edge_index = np.asarray(inputs["edge_index"])
    p = _host_prep(x, edge_index)

    consts = dict(
        att1_b=float(np.asarray(inputs["att1_b"]).reshape(-1)[0]),
        att2_b=float(np.asarray(inputs["att2_b"]).reshape(-1)[0]),
    )
    key = (tuple(p["CA"]), tuple(p["CB"]), consts["att1_b"], consts["att2_b"])
    if key not in _CACHE:
        _CACHE[key] = _build_program(p, consts)
    nc = _CACHE[key]

    w4 = np.concatenate(
        [
            np.asarray(inputs["att1_w"], dtype=np.float32).reshape(2, HID).T,
            np.asarray(inputs["att2_w"], dtype=np.float32).reshape(2, HID).T,
        ],
        axis=1,
    )  # [HID, 4] = [wi1, wj1, wi2, wj2]
    common = dict(
        encw=np.ascontiguousarray(np.asarray(inputs["enc_w"], dtype=np.float32)),
        encb=np.asarray(inputs["enc_b"], dtype=np.float32).reshape(HID, 1),
        w4=np.ascontiguousarray(w4),
        clsw=np.ascontiguousarray(np.asarray(inputs["cls_w"], dtype=np.float32)),
        clsb=np.asarray(inputs["cls_b"], dtype=np.float32).reshape(1, 2),
        ident=np.eye(128, dtype=np.float32),
    )
    in_maps = []
    for c in range(N_CORES):
        in_maps.append(
            dict(
                xT=np.ascontiguousarray(p["xT"][c]),
                idxA=np.ascontiguousarray(p["idx_a16"][c]),
                idxB=np.ascontiguousarray(p["idx_b16"][c]),
                maskT=np.ascontiguousarray(p["mask"][c].astype(ml_dtypes.bfloat16)),
                invdeg=np.ascontiguousarray(p["invdeg"][c]),
                **common,
            )
        )

    res = bass_utils.run_bass_kernel_spmd(nc, in_maps, core_ids=list(range(N_CORES)))
    kernel.last_result = res

    N = p["N"]
    out = np.zeros((N, 2), dtype=np.float32)
    for c in range(N_CORES):
        m = p["owner"] == c
        out[m] = np.asarray(res.results[c]["logits"], dtype=np.float32)[
            p["slabrow"][m]
        ]
    return out



# revision 32
# speedup vs baseline: 1.4092x; 1.1959x over previous
"""CARE-GNN forward on 8 Trainium2 NeuronCores (Bass/Tile).

Strategy (dst-sharded message passing):
  - Nodes are sharded across 8 cores by dst range; each core owns all edges
    into its nodes, so segment sums/counts complete locally (no all-reduce).
  - Per layer, every core reads a full node table [h | pj'] (bf16, 512B rows)
    from HBM, rebuilt via AllGather of per-core slabs.
  - Edges are laid out slot-aligned: the k-th in-edge of the node at tile
    position p lives at partition p of chunk k. Aggregation is then a
    diag(alpha) matmul accumulating into PSUM, and pi[dst] is just the
    per-tile pi column (no per-edge pi gather).
  - att([h_i, h_j]) = h_i @ wi + h_j @ wj, so alpha needs only per-node
    projections: pi' = h @ wi + att_b stays in SBUF, pj = h @ wj rides in
    the gathered row.
  - dma_gather uses signed int16 indices, so the table is addressed in two
    regions split at the core-5 slab boundary (5/8 vs 3/8 of rows, both
    < 32768); gathers are issued as uniform segments of <= SEG chunks per
    region per tile, round-robined over the 4 SWDGE queues, with a deep
    (bufs=8) buffer ring so desc-gen / DMA / sigmoid / weight / accumulate
    pipeline across segments. Empty slots gather row 0 and are zeroed by a
    validity mask folded into alpha.
  - Within a core, nodes are packed into tiles sorted by region-A degree
    with a windowed region-B subsort (tight chunk counts in both regions);
    tiles are then processed in a greedily balanced order.
  - PSUM->SBUF copies ride the Scalar (Act) engine so the Vector queue only
    carries the msg-multiply path that recycles gather buffers.
"""

import os
import sys
import types

import numpy as np
import ml_dtypes

N_CORES = 8
HID = 128
ROW_W = 2 * HID  # bf16 elements per table row (512B): [h(128) | pj'(1) | pad]
REG_SPLIT_CORE = 5  # table rows of cores 0..4 are region A, 5..7 region B


def _install_axon_ntff_hook():
    """Best-effort shim so trace=True (BASS_TRACE=1) works under axon."""
    try:
        if "antenv.axon_hooks" in sys.modules:
            return
        mod = types.ModuleType("antenv.axon_hooks")
        mod._hook = None
        mod.set_axon_ntff_profile_hook = lambda h: setattr(mod, "_hook", h)
        mod.get_axon_ntff_profile_hook = lambda: mod._hook
        sys.modules["antenv.axon_hooks"] = mod
        import antenv

        antenv.axon_hooks = mod
        from trn_agent_boot.trn_boot import _ntff_profile_via_ctypes

        so = "/opt/axon/libaxon_pjrt.so"
        if os.path.exists(so):
            mod.set_axon_ntff_profile_hook(_ntff_profile_via_ctypes(so))
    except Exception:
        pass


def _host_prep(x, edge_index):
    """Shard nodes/edges, build slot-aligned gather indices. Pure index work."""
    N = x.shape[0]
    src = np.asarray(edge_index[0], dtype=np.int64)
    dst = np.asarray(edge_index[1], dtype=np.int64)
    npc = (N + N_CORES - 1) // N_CORES  # nodes per core
    tpc = (npc + 127) // 128  # tiles per core
    slab = tpc * 128

    gtab = N_CORES * slab
    base_b = gtab - 32768  # region-B base row; region A covers rows [0, 32768)

    deg = np.bincount(dst, minlength=N)
    owner = np.minimum(np.arange(N) // npc, N_CORES - 1)
    # overlapping int16 regions: sources from cores 0-2 are A-only, cores
    # 5-7 B-only, cores 3-4 sit inside both regions and flex to either call
    cls_of_core = np.array([0, 0, 0, 2, 2, 1, 1, 1])
    e_cls = cls_of_core[owner[src]]
    adeg = np.bincount(dst[e_cls == 0], minlength=N)
    bdeg = np.bincount(dst[e_cls == 1], minlength=N)

    # node -> slab row: tiles packed by total degree desc (flex edges absorb
    # the split slack), windowed subsort by A-forced degree
    slabrow = np.empty(N, dtype=np.int64)
    for c in range(N_CORES):
        lo, hi = c * npc, min((c + 1) * npc, N)
        ad, td = adeg[lo:hi], deg[lo:hi]
        order = np.lexsort((-ad, -td))
        W = 512
        for s in range(0, hi - lo, W):
            w = order[s : s + W]
            order[s : s + W] = w[np.argsort(-ad[w], kind="stable")]
        slabrow[lo + order] = np.arange(hi - lo)
    table_row = owner * slab + slabrow

    e_src_row = table_row[src]
    e_core = owner[dst]
    e_slabrow = slabrow[dst]
    e_tile = e_slabrow // 128
    e_pos = e_slabrow % 128

    # per-tile common (ca, cb): feasible iff ca >= max a_p, cb >= max b_p,
    # ca+cb >= max tot_p over every core's 128 positions
    pa = np.zeros((N_CORES, tpc, 128), np.int64)
    pb = np.zeros_like(pa)
    pt = np.zeros_like(pa)
    np.add.at(pa, (e_core[e_cls == 0], e_tile[e_cls == 0], e_pos[e_cls == 0]), 1)
    np.add.at(pb, (e_core[e_cls == 1], e_tile[e_cls == 1], e_pos[e_cls == 1]), 1)
    np.add.at(pt, (e_core, e_tile, e_pos), 1)
    ca_t = pa.max(axis=2).max(axis=0)
    cb_t = pb.max(axis=2).max(axis=0)
    ct_t = np.maximum(pt.max(axis=2).max(axis=0), ca_t + cb_t)
    # split each tile's budget as evenly as feasibility allows so the A and
    # B gather calls (and their SWDGE queues) carry equal work
    CA_phys = np.clip(ct_t // 2, ca_t, ct_t - cb_t).astype(int)
    CB_phys = (ct_t - CA_phys).astype(int)
    assert (CA_phys + CB_phys).min() >= 1

    # each node sends min(f_p, CA - a_p) of its flex edges to the A call
    fa_cap = np.minimum(pt - pa - pb, np.maximum(CA_phys[None, :, None] - pa, 0))
    flex = e_cls == 2
    fkey = ((e_core * tpc + e_tile) * 128 + e_pos).astype(np.int64)
    forder = np.argsort(fkey[flex], kind="stable")
    fk = fkey[flex][forder]
    gs = np.r_[0, np.flatnonzero(np.diff(fk)) + 1]
    gl = np.diff(np.r_[gs, len(fk)])
    frank = np.empty(len(fk), dtype=np.int64)
    frank[forder] = np.arange(len(fk)) - np.repeat(gs, gl)
    e_reg = np.where(e_cls == 1, 1, 0)
    fidx = np.flatnonzero(flex)
    e_reg[fidx] = (
        frank >= fa_cap[e_core[fidx], e_tile[fidx], e_pos[fidx]]
    ).astype(np.int64)

    # per-(core,tile,pos,region) sequence number -> chunk index
    key = (((e_core * tpc + e_tile) * 128 + e_pos) * 2 + e_reg).astype(np.int64)
    order = np.argsort(key, kind="stable")
    ks = key[order]
    grp_start = np.r_[0, np.flatnonzero(np.diff(ks)) + 1]
    grp_len = np.diff(np.r_[grp_start, len(ks)])
    seq = np.arange(len(ks)) - np.repeat(grp_start, grp_len)
    e_seq = np.empty(len(ks), dtype=np.int64)
    e_seq[order] = seq

    # verify the region assignment stays within the per-tile chunk budget
    cnt = np.zeros((N_CORES, tpc, 128, 2), dtype=np.int64)
    np.add.at(cnt, (e_core, e_tile, e_pos, e_reg), 1)
    c_reg = cnt.max(axis=2).max(axis=0)
    assert (c_reg[:, 0] <= CA_phys).all() and (c_reg[:, 1] <= CB_phys).all()

    # processing order: tiles greedily paired into groups of GRP so each
    # group's chunk totals are balanced (gathers are per group+region)
    GRP = 2
    ngrp = (tpc + GRP - 1) // GRP
    torder = np.argsort(-(CA_phys + CB_phys), kind="stable")
    gsum = np.zeros(ngrp)
    gcnt = np.zeros(ngrp, dtype=int)
    groups = [[] for _ in range(ngrp)]
    for t in torder:
        cand = [i for i in range(ngrp) if gcnt[i] < GRP]
        i = min(cand, key=lambda i: gsum[i])
        groups[i].append(int(t))
        gsum[i] += CA_phys[t] + CB_phys[t]
        gcnt[i] += 1
    proc = [t for g in groups for t in g]  # processing slot -> physical tile
    slot_of = np.empty(tpc, dtype=np.int64)
    slot_of[proc] = np.arange(tpc)

    CA = CA_phys[proc]
    CB = CB_phys[proc]
    CT = CA + CB
    offs_a = np.r_[0, np.cumsum(CA)].astype(int)
    offs_b = np.r_[0, np.cumsum(CB)].astype(int)
    offs_t = np.r_[0, np.cumsum(CT)].astype(int)
    tot_a, tot_b, tot_t = int(offs_a[-1]), int(offs_b[-1]), int(offs_t[-1])

    # gather index arrays in PROCESSING order (slot = chunk*128 + pos)
    e_slot = slot_of[e_tile]
    # padding slots gather the hole row (slabrow 6250 of core 0 / core 7)
    # whose stored pj' is poisoned to -1e4, so sigmoid gives alpha = 0
    # exactly and no validity mask is needed
    pad_a = npc
    pad_b = (N_CORES - 1) * slab + npc - base_b
    assert 0 <= pad_a < 32768 and 0 <= pad_b < 32768 and npc < slab
    idx_a = np.full((N_CORES, max(tot_a, 1) * 128), pad_a, dtype=np.int64)
    idx_b = np.full((N_CORES, max(tot_b, 1) * 128), pad_b, dtype=np.int64)

    e_off = np.where(e_reg == 0, offs_a[e_slot] * 128, offs_b[e_slot] * 128)
    e_lin = e_off + e_seq * 128 + e_pos
    e_val = np.where(e_reg == 0, e_src_row, e_src_row - base_b)
    assert e_val.min() >= 0 and e_val.max() < 32768
    for c in range(N_CORES):
        m = e_core == c
        ra = m & (e_reg == 0)
        rb = m & (e_reg == 1)
        idx_a[c, e_lin[ra]] = e_val[ra]
        idx_b[c, e_lin[rb]] = e_val[rb]

    def wrap16(lin):  # [n] -> [128, n//16] int16 (16-part wrap, replicated x8)
        w = lin.reshape(-1, 16).T.astype(np.uint16).view(np.int16)  # [16, n/16]
        return np.tile(w, (8, 1))

    idx_a16 = np.stack([wrap16(idx_a[c]) for c in range(N_CORES)])
    idx_b16 = np.stack([wrap16(idx_b[c]) for c in range(N_CORES)])

    # inverse degree, laid out [pos, tile]; holes -> 1.0
    invdeg = np.ones((N_CORES, slab), dtype=np.float32)
    invdeg[owner, slabrow] = 1.0 / np.maximum(deg, 1).astype(np.float32)
    invdeg = invdeg.reshape(N_CORES, tpc, 128).transpose(0, 2, 1).copy()

    # x slabs, transposed: [in_dim, slab] per core
    in_dim = x.shape[1]
    xT = np.zeros((N_CORES, in_dim, slab), dtype=np.float32)
    for c in range(N_CORES):
        lo, hi = c * npc, min((c + 1) * npc, N)
        xT[c][:, slabrow[lo:hi]] = np.asarray(x[lo:hi], dtype=np.float32).T

    ga_max = max(
        int(offs_a[min(i0 + GRP, tpc)] - offs_a[i0]) for i0 in range(0, tpc, GRP)
    )
    gb_max = max(
        int(offs_b[min(i0 + GRP, tpc)] - offs_b[i0]) for i0 in range(0, tpc, GRP)
    )
    return dict(
        N=N, npc=npc, tpc=tpc, slab=slab, half=base_b, in_dim=in_dim,
        CA=CA, CB=CB, offs_a=offs_a, offs_b=offs_b, offs_t=offs_t,
        tot_a=tot_a, tot_b=tot_b, tot_t=tot_t,
        idx_a16=idx_a16, idx_b16=idx_b16, invdeg=invdeg, xT=xT,
        owner=owner, slabrow=slabrow,
        proc=proc, GRP=GRP, ga_max=ga_max, gb_max=gb_max,
    )


def _build_program(p, consts):
    import concourse.bacc as bacc
    import concourse.mybir as mybir
    import concourse.tile as tile

    f32 = mybir.dt.float32
    bf16 = mybir.dt.bfloat16
    i16 = mybir.dt.int16
    AF = mybir.ActivationFunctionType

    tpc, slab, in_dim, half = p["tpc"], p["slab"], p["in_dim"], p["half"]
    CA, CB = p["CA"], p["CB"]
    offs_a, offs_b, offs_t = p["offs_a"], p["offs_b"], p["offs_t"]
    tot_a, tot_b, tot_t = p["tot_a"], p["tot_b"], p["tot_t"]
    att_bs = (consts["att1_b"], consts["att2_b"])
    gtab = N_CORES * slab
    nk = in_dim // 128  # contraction tiles for the encoder
    SEG = 16  # msg build granularity in chunks
    proc, GRP = p["proc"], p["GRP"]
    ga_max, gb_max = p["ga_max"], p["gb_max"]

    nc = bacc.Bacc("TRN2", num_devices=N_CORES, num_swdge_queues=4, dynamic_dma_scratch_size=49152)

    # ---- I/O ----
    xT = nc.dram_tensor("xT", [in_dim, slab], f32, kind="ExternalInput")
    idxA = nc.dram_tensor("idxA", [128, max(tot_a, 1) * 8], i16, kind="ExternalInput")
    idxB = nc.dram_tensor("idxB", [128, max(tot_b, 1) * 8], i16, kind="ExternalInput")
    invdeg = nc.dram_tensor("invdeg", [128, tpc], f32, kind="ExternalInput")
    encw = nc.dram_tensor("encw", [in_dim, HID], f32, kind="ExternalInput")
    encb = nc.dram_tensor("encb", [HID, 1], f32, kind="ExternalInput")
    w4 = nc.dram_tensor("w4", [HID, 4], f32, kind="ExternalInput")
    clsw = nc.dram_tensor("clsw", [HID, 2], f32, kind="ExternalInput")
    clsb = nc.dram_tensor("clsb", [1, 2], f32, kind="ExternalInput")
    ident_in = nc.dram_tensor("ident", [128, 128], f32, kind="ExternalInput")
    logits = nc.dram_tensor("logits", [slab, 2], f32, kind="ExternalOutput")

    # ---- internal DRAM ----
    slabs = [nc.dram_tensor(f"slab{l}", [slab, ROW_W], bf16) for l in (1, 2)]
    tables = [
        nc.dram_tensor(f"table{l}", [gtab, ROW_W], bf16, addr_space="Shared")
        for l in (1, 2)
    ]

    with tile.TileContext(nc) as tc:
        with (
            tc.tile_pool(name="const", bufs=1) as cpool,
            tc.tile_pool(name="work", bufs=3) as pool,
            tc.tile_pool(name="dias", bufs=4) as dpool,
            tc.tile_pool(name="gath", bufs=8) as gpool,
            tc.tile_pool(name="psacc", bufs=3, space="PSUM") as ps_acc,
            tc.tile_pool(name="pstr", bufs=3, space="PSUM") as ps_tr,
            tc.tile_pool(name="pssm", bufs=2, space="PSUM") as ps_sm,
        ):
            # ---- constants / whole-kernel residents in SBUF ----
            encw_t = [cpool.tile([128, HID], f32, tag=f"encw{i}", name=f"encw{i}") for i in range(nk)]
            for i, t in enumerate(encw_t):
                nc.sync.dma_start(out=t[:], in_=encw[i * 128 : (i + 1) * 128, :])
            encb_t = cpool.tile([HID, 1], f32, tag="encb")
            nc.sync.dma_start(out=encb_t[:], in_=encb[:])
            w4_t = cpool.tile([HID, 4], f32, tag="w4")
            nc.sync.dma_start(out=w4_t[:], in_=w4[:])
            clsw_t = cpool.tile([HID, 2], f32, tag="clsw")
            nc.sync.dma_start(out=clsw_t[:], in_=clsw[:])
            clsb_t = cpool.tile([1, 2], f32, tag="clsb")
            nc.sync.dma_start(out=clsb_t[:], in_=clsb[:])
            ident_f = cpool.tile([128, 128], f32, tag="identf")
            nc.sync.dma_start(out=ident_f[:], in_=ident_in[:])
            ident_b = cpool.tile([128, 128], bf16, tag="identb")
            nc.vector.tensor_copy(out=ident_b[:], in_=ident_f[:])
            ones_f = cpool.tile([1, 128], f32, tag="onesf")
            nc.vector.memset(ones_f[:], 1.0)
            inv_all = cpool.tile([128, tpc], f32, tag="invall")
            nc.sync.dma_start(out=inv_all[:], in_=invdeg[:])
            idxA_t = cpool.tile([128, max(tot_a, 1) * 8], i16, tag="idxAt")
            nc.sync.dma_start(out=idxA_t[:], in_=idxA[:])
            idxB_t = cpool.tile([128, max(tot_b, 1) * 8], i16, tag="idxBt")
            nc.sync.dma_start(out=idxB_t[:], in_=idxB[:])
            poison_t = cpool.tile([1, 1], bf16, tag="poison")
            nc.vector.memset(poison_t[:], -10000.0)
            pi_all = [
                cpool.tile([128, tpc], f32, tag=f"piall{l}", name=f"piall{l}")
                for l in (1, 2)
            ]

            def p_phase_and_store(hT_sb, t, layer):
                """hT (f32 [hid, nodes]) -> slab rows [h|pj'] + pi' column."""
                co = t * 128
                lw = 2 * (layer - 1)
                h_ps = ps_tr.tile([128, 128], f32, tag="tr")
                nc.tensor.transpose(out=h_ps[:], in_=hT_sb[:], identity=ident_f[:])
                h_sb = pool.tile([128, 128], bf16, tag="hsb")
                nc.vector.tensor_copy(out=h_sb[:], in_=h_ps[:])
                nc.sync.dma_start(
                    out=slabs[layer - 1][co : co + 128, 0:HID], in_=h_sb[:]
                )
                p_ps = ps_sm.tile([128, 2], f32, tag="sm")
                nc.tensor.matmul(
                    out=p_ps[:], lhsT=hT_sb[:], rhs=w4_t[:, lw : lw + 2],
                    start=True, stop=True,
                )
                # pi' = pi + att_b kept in SBUF for the sigmoid bias
                nc.scalar.add(
                    out=pi_all[layer - 1][:, t : t + 1],
                    in_=p_ps[:, 0:1],
                    add=float(att_bs[layer - 1]),
                )
                pj_ext = pool.tile([128, ROW_W - HID], bf16, tag="pjext")
                nc.vector.memset(pj_ext[:], 0.0)
                nc.vector.tensor_copy(out=pj_ext[:, 0:1], in_=p_ps[:, 1:2])
                nc.sync.dma_start(
                    out=slabs[layer - 1][co : co + 128, HID:ROW_W], in_=pj_ext[:]
                )

            # ---- encoder: hT = relu(encw.T @ xT + encb), then p1 ----
            for t in range(tpc):
                co = t * 128
                xt = [pool.tile([128, 128], f32, tag=f"xt{i}", name=f"xt{i}") for i in range(nk)]
                for i, xx in enumerate(xt):
                    nc.sync.dma_start(
                        out=xx[:], in_=xT[i * 128 : (i + 1) * 128, co : co + 128]
                    )
                hT_ps = ps_tr.tile([128, 128], f32, tag="tr")
                for i in range(nk):
                    nc.tensor.matmul(
                        out=hT_ps[:], lhsT=encw_t[i][:], rhs=xt[i][:],
                        start=(i == 0), stop=(i == nk - 1),
                    )
                hT_sb = pool.tile([128, 128], f32, tag="hTsb")
                nc.scalar.activation(
                    out=hT_sb[:], in_=hT_ps[:], func=AF.Relu, bias=encb_t[:]
                )
                p_phase_and_store(hT_sb, t, layer=1)

            # ---- two message-passing layers ----
            gather_ctr = [0]
            for layer in (1, 2):
                table = tables[layer - 1]
                # poison the hole row's pj' so padding slots get alpha = 0
                nc.sync.dma_start(
                    out=slabs[layer - 1][p["npc"] : p["npc"] + 1, HID : HID + 1],
                    in_=poison_t[:],
                )
                nc.gpsimd.collective_compute(
                    "AllGather",
                    mybir.AluOpType.bypass,
                    replica_groups=[list(range(N_CORES))],
                    ins=[slabs[layer - 1][:]],
                    outs=[table[:]],
                )
                for i in range(tpc):
                    t = proc[i]
                    co = t * 128
                    ca, cb = int(CA[i]), int(CB[i])
                    ct = ca + cb
                    # uniform gather segments of <= SEG chunks, each segment
                    # gathered / sigmoided / weighted / accumulated on its own
                    # so its buffer recycles immediately
                    segs = []
                    for c0 in range(0, ca, SEG):
                        segs.append((0, c0, min(SEG, ca - c0), c0))
                    for c0 in range(0, cb, SEG):
                        segs.append((1, c0, min(SEG, cb - c0), ca + c0))
                    alpha = pool.tile([128, ct], bf16, tag="alpha")
                    acc = ps_acc.tile([128, HID], f32, tag="acc")
                    pi_col = pi_all[layer - 1][:, t : t + 1]
                    for si, (reg, c0, cw, aoff) in enumerate(segs):
                        idx_t, offs, base, rows = (
                            (idxA_t, offs_a, 0, 32768)
                            if reg == 0
                            else (idxB_t, offs_b, gtab - 32768, 32768)
                        )
                        gt = gpool.tile([128, SEG, ROW_W], bf16, tag="g", name="g")
                        nc.gpsimd.dma_gather(
                            out_ap=gt[:, 0:cw, :],
                            in_ap=table[base : base + rows, :],
                            idxs_ap=idx_t[
                                :, (offs[i] + c0) * 8 : (offs[i] + c0 + cw) * 8
                            ],
                            num_idxs=cw * 128,
                            num_idxs_reg=cw * 128,
                            elem_size=ROW_W,
                            single_packet=False,
                            queue_num=gather_ctr[0] % 4,
                        )
                        gather_ctr[0] += 1
                        # alpha = sigmoid(pj + pi') * mask for this segment
                        nc.scalar.activation(
                            out=alpha[:, aoff : aoff + cw, None],
                            in_=gt[:, 0:cw, HID : HID + 1],
                            func=AF.Sigmoid,
                            bias=pi_col,
                        )
                        # msg = alpha (*) rows; slot-sum via identity-matmul
                        # PSUM accumulation (acc[pos] += msg[pos, k, :])
                        msg = dpool.tile([128, SEG, 128], bf16, tag="msg")
                        nc.vector.tensor_tensor(
                            out=msg[:, 0:cw, :],
                            in0=gt[:, 0:cw, 0:HID],
                            in1=alpha[:, aoff : aoff + cw, None].to_broadcast(
                                [128, cw, 128]
                            ),
                            op=mybir.AluOpType.mult,
                        )
                        for k in range(cw):
                            nc.tensor.matmul(
                                out=acc[:],
                                lhsT=ident_b[:],
                                rhs=msg[:, k, :],
                                start=(si == 0 and k == 0),
                                stop=(si == len(segs) - 1 and k == cw - 1),
                            )
                    inv_col = inv_all[:, t : t + 1]
                        if layer == 1:
                            # h2 = relu(acc * invdeg); p2 phase + stores
                            h2_sb = pool.tile([128, 128], f32, tag="h2sb")
                            nc.scalar.activation(
                                out=h2_sb[:], in_=acc[:], func=AF.Relu, scale=inv_col
                            )
                            hT2_ps = ps_tr.tile([128, 128], f32, tag="tr")
                            nc.tensor.transpose(
                                out=hT2_ps[:], in_=h2_sb[:], identity=ident_f[:]
                            )
                            hT2_sb = pool.tile([128, 128], f32, tag="hT2sb")
                            nc.vector.tensor_copy(out=hT2_sb[:], in_=hT2_ps[:])
                            p_phase_and_store(hT2_sb, t, layer=2)
                        else:
                            # logits = (acc * invdeg) @ clsw + clsb
                            m_sb = pool.tile([128, 128], f32, tag="msb")
                            nc.scalar.mul(out=m_sb[:], in_=acc[:], mul=inv_col)
                            mT_ps = ps_tr.tile([128, 128], f32, tag="tr")
                            nc.tensor.transpose(
                                out=mT_ps[:], in_=m_sb[:], identity=ident_f[:]
                            )
                            mT_sb = pool.tile([128, 128], f32, tag="mTsb")
                            nc.vector.tensor_copy(out=mT_sb[:], in_=mT_ps[:])
                            lg_ps = ps_sm.tile([128, 2], f32, tag="sm")
                            nc.tensor.matmul(
                                out=lg_ps[:], lhsT=mT_sb[:], rhs=clsw_t[:],
                                start=True, stop=False,
                            )
                            nc.tensor.matmul(
                                out=lg_ps[:], lhsT=ones_f[:], rhs=clsb_t[:],
                                start=False, stop=True,
                            )
                            lg_sb = pool.tile([128, 2], f32, tag="lgsb")
                            nc.vector.tensor_copy(out=lg_sb[:], in_=lg_ps[:])
                            nc.sync.dma_start(
                                out=logits[co : co + 128, :], in_=lg_sb[:]
                            )

    nc.compile()
    return nc


_CACHE = {}


def kernel(**inputs):
    _install_axon_ntff_hook()
    from concourse import bass_utils

    bass_utils.upload_artifacts = lambda tmpdir: tmpdir

    x = np.asarray(inputs["x"], dtype=np.float32)
    edge_index = np.asarray(inputs["edge_index"])
    p = _host_prep(x, edge_index)

    consts = dict(
        att1_b=float(np.asarray(inputs["att1_b"]).reshape(-1)[0]),
        att2_b=float(np.asarray(inputs["att2_b"]).reshape(-1)[0]),
    )
    key = (tuple(p["CA"]), tuple(p["CB"]), consts["att1_b"], consts["att2_b"])
    if key not in _CACHE:
        _CACHE[key] = _build_program(p, consts)
    nc = _CACHE[key]

    w4 = np.concatenate(
        [
            np.asarray(inputs["att1_w"], dtype=np.float32).reshape(2, HID).T,
            np.asarray(inputs["att2_w"], dtype=np.float32).reshape(2, HID).T,
        ],
        axis=1,
    )  # [HID, 4] = [wi1, wj1, wi2, wj2]
    common = dict(
        encw=np.ascontiguousarray(np.asarray(inputs["enc_w"], dtype=np.float32)),
        encb=np.asarray(inputs["enc_b"], dtype=np.float32).reshape(HID, 1),
        w4=np.ascontiguousarray(w4),
        clsw=np.ascontiguousarray(np.asarray(inputs["cls_w"], dtype=np.float32)),
        clsb=np.asarray(inputs["cls_b"], dtype=np.float32).reshape(1, 2),
        ident=np.eye(128, dtype=np.float32),
    )
    in_maps = []
    for c in range(N_CORES):
        in_maps.append(
            dict(
                xT=np.ascontiguousarray(p["xT"][c]),
                idxA=np.ascontiguousarray(p["idx_a16"][c]),
                idxB=np.ascontiguousarray(p["idx_b16"][c]),
                invdeg=np.ascontiguousarray(p["invdeg"][c]),
                **common,
            )
        )

    res = bass_utils.run_bass_kernel_spmd(nc, in_maps, core_ids=list(range(N_CORES)))
    kernel.last_result = res

    N = p["N"]
    out = np.zeros((N, 2), dtype=np.float32)
    for c in range(N_CORES):
        m = p["owner"] == c
        out[m] = np.asarray(res.results[c]["logits"], dtype=np.float32)[
            p["slabrow"][m]
        ]
    return out



# revision 33
# speedup vs baseline: 1.4961x; 1.0617x over previous
"""CARE-GNN forward on 8 Trainium2 NeuronCores (Bass/Tile).

Strategy (dst-sharded message passing):
  - Nodes are sharded across 8 cores by dst range; each core owns all edges
    into its nodes, so segment sums/counts complete locally (no all-reduce).
  - Per layer, every core reads a full node table [h | pj'] (bf16, 512B rows)
    from HBM, rebuilt via AllGather of per-core slabs.
  - Edges are laid out slot-aligned: the k-th in-edge of the node at tile
    position p lives at partition p of chunk k. Aggregation is then a
    diag(alpha) matmul accumulating into PSUM, and pi[dst] is just the
    per-tile pi column (no per-edge pi gather).
  - att([h_i, h_j]) = h_i @ wi + h_j @ wj, so alpha needs only per-node
    projections: pi' = h @ wi + att_b stays in SBUF, pj = h @ wj rides in
    the gathered row.
  - dma_gather uses signed int16 indices, so the table is addressed in two
    regions split at the core-5 slab boundary (5/8 vs 3/8 of rows, both
    < 32768); gathers are issued as uniform segments of <= SEG chunks per
    region per tile, round-robined over the 4 SWDGE queues, with a deep
    (bufs=8) buffer ring so desc-gen / DMA / sigmoid / weight / accumulate
    pipeline across segments. Empty slots gather row 0 and are zeroed by a
    validity mask folded into alpha.
  - Within a core, nodes are packed into tiles sorted by region-A degree
    with a windowed region-B subsort (tight chunk counts in both regions);
    tiles are then processed in a greedily balanced order.
  - PSUM->SBUF copies ride the Scalar (Act) engine so the Vector queue only
    carries the msg-multiply path that recycles gather buffers.
"""

import os
import sys
import types

import numpy as np
import ml_dtypes

N_CORES = 8
HID = 128
ROW_W = 2 * HID  # bf16 elements per table row (512B): [h(128) | pj'(1) | pad]
REG_SPLIT_CORE = 5  # table rows of cores 0..4 are region A, 5..7 region B


def _install_axon_ntff_hook():
    """Best-effort shim so trace=True (BASS_TRACE=1) works under axon."""
    try:
        if "antenv.axon_hooks" in sys.modules:
            return
        mod = types.ModuleType("antenv.axon_hooks")
        mod._hook = None
        mod.set_axon_ntff_profile_hook = lambda h: setattr(mod, "_hook", h)
        mod.get_axon_ntff_profile_hook = lambda: mod._hook
        sys.modules["antenv.axon_hooks"] = mod
        import antenv

        antenv.axon_hooks = mod
        from trn_agent_boot.trn_boot import _ntff_profile_via_ctypes

        so = "/opt/axon/libaxon_pjrt.so"
        if os.path.exists(so):
            mod.set_axon_ntff_profile_hook(_ntff_profile_via_ctypes(so))
    except Exception:
        pass


def _host_prep(x, edge_index):
    """Shard nodes/edges, build slot-aligned gather indices. Pure index work."""
    N = x.shape[0]
    src = np.asarray(edge_index[0], dtype=np.int64)
    dst = np.asarray(edge_index[1], dtype=np.int64)
    npc = (N + N_CORES - 1) // N_CORES  # nodes per core
    tpc = (npc + 127) // 128  # tiles per core
    slab = tpc * 128

    gtab = N_CORES * slab
    base_b = gtab - 32768  # region-B base row; region A covers rows [0, 32768)

    deg = np.bincount(dst, minlength=N)
    owner = np.minimum(np.arange(N) // npc, N_CORES - 1)
    # overlapping int16 regions: sources from cores 0-2 are A-only, cores
    # 5-7 B-only, cores 3-4 sit inside both regions and flex to either call
    cls_of_core = np.array([0, 0, 0, 2, 2, 1, 1, 1])
    e_cls = cls_of_core[owner[src]]
    adeg = np.bincount(dst[e_cls == 0], minlength=N)
    bdeg = np.bincount(dst[e_cls == 1], minlength=N)

    # node -> slab row: tiles packed by total degree desc (flex edges absorb
    # the split slack), windowed subsort by A-forced degree
    slabrow = np.empty(N, dtype=np.int64)
    for c in range(N_CORES):
        lo, hi = c * npc, min((c + 1) * npc, N)
        ad, td = adeg[lo:hi], deg[lo:hi]
        order = np.lexsort((-ad, -td))
        W = 512
        for s in range(0, hi - lo, W):
            w = order[s : s + W]
            order[s : s + W] = w[np.argsort(-ad[w], kind="stable")]
        slabrow[lo + order] = np.arange(hi - lo)
    table_row = owner * slab + slabrow

    e_src_row = table_row[src]
    e_core = owner[dst]
    e_slabrow = slabrow[dst]
    e_tile = e_slabrow // 128
    e_pos = e_slabrow % 128
    # refine classes at row granularity: every source row inside the overlap
    # band [base_b, 32768) can flex to either call
    e_cls = np.where(e_src_row < base_b, 0, np.where(e_src_row >= 32768, 1, 2))

    # per-tile common (ca, cb): feasible iff ca >= max a_p, cb >= max b_p,
    # ca+cb >= max tot_p over every core's 128 positions
    pa = np.zeros((N_CORES, tpc, 128), np.int64)
    pb = np.zeros_like(pa)
    pt = np.zeros_like(pa)
    np.add.at(pa, (e_core[e_cls == 0], e_tile[e_cls == 0], e_pos[e_cls == 0]), 1)
    np.add.at(pb, (e_core[e_cls == 1], e_tile[e_cls == 1], e_pos[e_cls == 1]), 1)
    np.add.at(pt, (e_core, e_tile, e_pos), 1)
    ca_t = pa.max(axis=2).max(axis=0)
    cb_t = pb.max(axis=2).max(axis=0)
    ct_t = np.maximum(pt.max(axis=2).max(axis=0), ca_t + cb_t)
    # split each tile's budget as evenly as feasibility allows so the A and
    # B gather calls (and their SWDGE queues) carry equal work
    CA_phys = np.clip(ct_t // 2, ca_t, ct_t - cb_t).astype(int)
    CB_phys = (ct_t - CA_phys).astype(int)
    assert (CA_phys + CB_phys).min() >= 1

    # each node sends min(f_p, CA - a_p) of its flex edges to the A call
    fa_cap = np.minimum(pt - pa - pb, np.maximum(CA_phys[None, :, None] - pa, 0))
    flex = e_cls == 2
    fkey = ((e_core * tpc + e_tile) * 128 + e_pos).astype(np.int64)
    forder = np.argsort(fkey[flex], kind="stable")
    fk = fkey[flex][forder]
    gs = np.r_[0, np.flatnonzero(np.diff(fk)) + 1]
    gl = np.diff(np.r_[gs, len(fk)])
    frank = np.empty(len(fk), dtype=np.int64)
    frank[forder] = np.arange(len(fk)) - np.repeat(gs, gl)
    e_reg = np.where(e_cls == 1, 1, 0)
    fidx = np.flatnonzero(flex)
    e_reg[fidx] = (
        frank >= fa_cap[e_core[fidx], e_tile[fidx], e_pos[fidx]]
    ).astype(np.int64)

    # per-(core,tile,pos,region) sequence number -> chunk index
    key = (((e_core * tpc + e_tile) * 128 + e_pos) * 2 + e_reg).astype(np.int64)
    order = np.argsort(key, kind="stable")
    ks = key[order]
    grp_start = np.r_[0, np.flatnonzero(np.diff(ks)) + 1]
    grp_len = np.diff(np.r_[grp_start, len(ks)])
    seq = np.arange(len(ks)) - np.repeat(grp_start, grp_len)
    e_seq = np.empty(len(ks), dtype=np.int64)
    e_seq[order] = seq

    # verify the region assignment stays within the per-tile chunk budget
    cnt = np.zeros((N_CORES, tpc, 128, 2), dtype=np.int64)
    np.add.at(cnt, (e_core, e_tile, e_pos, e_reg), 1)
    c_reg = cnt.max(axis=2).max(axis=0)
    assert (c_reg[:, 0] <= CA_phys).all() and (c_reg[:, 1] <= CB_phys).all()

    # processing order: tiles greedily paired into groups of GRP so each
    # group's chunk totals are balanced (gathers are per group+region)
    GRP = 2
    ngrp = (tpc + GRP - 1) // GRP
    torder = np.argsort(-(CA_phys + CB_phys), kind="stable")
    gsum = np.zeros(ngrp)
    gcnt = np.zeros(ngrp, dtype=int)
    groups = [[] for _ in range(ngrp)]
    for t in torder:
        cand = [i for i in range(ngrp) if gcnt[i] < GRP]
        i = min(cand, key=lambda i: gsum[i])
        groups[i].append(int(t))
        gsum[i] += CA_phys[t] + CB_phys[t]
        gcnt[i] += 1
    proc = [t for g in groups for t in g]  # processing slot -> physical tile
    slot_of = np.empty(tpc, dtype=np.int64)
    slot_of[proc] = np.arange(tpc)

    CA = CA_phys[proc]
    CB = CB_phys[proc]
    CT = CA + CB
    offs_a = np.r_[0, np.cumsum(CA)].astype(int)
    offs_b = np.r_[0, np.cumsum(CB)].astype(int)
    offs_t = np.r_[0, np.cumsum(CT)].astype(int)
    tot_a, tot_b, tot_t = int(offs_a[-1]), int(offs_b[-1]), int(offs_t[-1])

    # gather index arrays in PROCESSING order (slot = chunk*128 + pos)
    e_slot = slot_of[e_tile]
    # padding slots gather the hole row (slabrow 6250 of core 0 / core 7)
    # whose stored pj' is poisoned to -1e4, so sigmoid gives alpha = 0
    # exactly and no validity mask is needed
    pad_a = npc
    pad_b = (N_CORES - 1) * slab + npc - base_b
    assert 0 <= pad_a < 32768 and 0 <= pad_b < 32768 and npc < slab
    idx_a = np.full((N_CORES, max(tot_a, 1) * 128), pad_a, dtype=np.int64)
    idx_b = np.full((N_CORES, max(tot_b, 1) * 128), pad_b, dtype=np.int64)

    e_off = np.where(e_reg == 0, offs_a[e_slot] * 128, offs_b[e_slot] * 128)
    e_lin = e_off + e_seq * 128 + e_pos
    e_val = np.where(e_reg == 0, e_src_row, e_src_row - base_b)
    assert e_val.min() >= 0 and e_val.max() < 32768
    for c in range(N_CORES):
        m = e_core == c
        ra = m & (e_reg == 0)
        rb = m & (e_reg == 1)
        idx_a[c, e_lin[ra]] = e_val[ra]
        idx_b[c, e_lin[rb]] = e_val[rb]

    def wrap16(lin):  # [n] -> [128, n//16] int16 (16-part wrap, replicated x8)
        w = lin.reshape(-1, 16).T.astype(np.uint16).view(np.int16)  # [16, n/16]
        return np.tile(w, (8, 1))

    idx_a16 = np.stack([wrap16(idx_a[c]) for c in range(N_CORES)])
    idx_b16 = np.stack([wrap16(idx_b[c]) for c in range(N_CORES)])

    # inverse degree, laid out [pos, tile]; holes -> 1.0
    invdeg = np.ones((N_CORES, slab), dtype=np.float32)
    invdeg[owner, slabrow] = 1.0 / np.maximum(deg, 1).astype(np.float32)
    invdeg = invdeg.reshape(N_CORES, tpc, 128).transpose(0, 2, 1).copy()

    # x slabs, transposed: [in_dim, slab] per core
    in_dim = x.shape[1]
    xT = np.zeros((N_CORES, in_dim, slab), dtype=np.float32)
    for c in range(N_CORES):
        lo, hi = c * npc, min((c + 1) * npc, N)
        xT[c][:, slabrow[lo:hi]] = np.asarray(x[lo:hi], dtype=np.float32).T

    ga_max = max(
        int(offs_a[min(i0 + GRP, tpc)] - offs_a[i0]) for i0 in range(0, tpc, GRP)
    )
    gb_max = max(
        int(offs_b[min(i0 + GRP, tpc)] - offs_b[i0]) for i0 in range(0, tpc, GRP)
    )
    return dict(
        N=N, npc=npc, tpc=tpc, slab=slab, half=base_b, in_dim=in_dim,
        CA=CA, CB=CB, offs_a=offs_a, offs_b=offs_b, offs_t=offs_t,
        tot_a=tot_a, tot_b=tot_b, tot_t=tot_t,
        idx_a16=idx_a16, idx_b16=idx_b16, invdeg=invdeg, xT=xT,
        owner=owner, slabrow=slabrow,
        proc=proc, GRP=GRP, ga_max=ga_max, gb_max=gb_max,
    )


def _build_program(p, consts):
    import concourse.bacc as bacc
    import concourse.mybir as mybir
    import concourse.tile as tile

    f32 = mybir.dt.float32
    bf16 = mybir.dt.bfloat16
    i16 = mybir.dt.int16
    AF = mybir.ActivationFunctionType

    tpc, slab, in_dim, half = p["tpc"], p["slab"], p["in_dim"], p["half"]
    CA, CB = p["CA"], p["CB"]
    offs_a, offs_b, offs_t = p["offs_a"], p["offs_b"], p["offs_t"]
    tot_a, tot_b, tot_t = p["tot_a"], p["tot_b"], p["tot_t"]
    att_bs = (consts["att1_b"], consts["att2_b"])
    gtab = N_CORES * slab
    nk = in_dim // 128  # contraction tiles for the encoder
    SEG = 16  # msg build granularity in chunks
    proc, GRP = p["proc"], p["GRP"]
    ga_max, gb_max = p["ga_max"], p["gb_max"]

    nc = bacc.Bacc("TRN2", num_devices=N_CORES, num_swdge_queues=4, dynamic_dma_scratch_size=49152)

    # ---- I/O ----
    xT = nc.dram_tensor("xT", [in_dim, slab], f32, kind="ExternalInput")
    idxA = nc.dram_tensor("idxA", [128, max(tot_a, 1) * 8], i16, kind="ExternalInput")
    idxB = nc.dram_tensor("idxB", [128, max(tot_b, 1) * 8], i16, kind="ExternalInput")
    invdeg = nc.dram_tensor("invdeg", [128, tpc], f32, kind="ExternalInput")
    encw = nc.dram_tensor("encw", [in_dim, HID], f32, kind="ExternalInput")
    encb = nc.dram_tensor("encb", [HID, 1], f32, kind="ExternalInput")
    w4 = nc.dram_tensor("w4", [HID, 4], f32, kind="ExternalInput")
    clsw = nc.dram_tensor("clsw", [HID, 2], f32, kind="ExternalInput")
    clsb = nc.dram_tensor("clsb", [1, 2], f32, kind="ExternalInput")
    ident_in = nc.dram_tensor("ident", [128, 128], f32, kind="ExternalInput")
    logits = nc.dram_tensor("logits", [slab, 2], f32, kind="ExternalOutput")

    # ---- internal DRAM ----
    slabs = [nc.dram_tensor(f"slab{l}", [slab, ROW_W], bf16) for l in (1, 2)]
    tables = [
        nc.dram_tensor(f"table{l}", [gtab, ROW_W], bf16, addr_space="Shared")
        for l in (1, 2)
    ]

    with tile.TileContext(nc) as tc:
        with (
            tc.tile_pool(name="const", bufs=1) as cpool,
            tc.tile_pool(name="work", bufs=3) as pool,
            tc.tile_pool(name="dias", bufs=6) as dpool,
            tc.tile_pool(name="gath", bufs=8) as gpool,
            tc.tile_pool(name="psacc", bufs=3, space="PSUM") as ps_acc,
            tc.tile_pool(name="pstr", bufs=3, space="PSUM") as ps_tr,
            tc.tile_pool(name="pssm", bufs=2, space="PSUM") as ps_sm,
        ):
            # ---- constants / whole-kernel residents in SBUF ----
            encw_t = [cpool.tile([128, HID], f32, tag=f"encw{i}", name=f"encw{i}") for i in range(nk)]
            for i, t in enumerate(encw_t):
                nc.sync.dma_start(out=t[:], in_=encw[i * 128 : (i + 1) * 128, :])
            encb_t = cpool.tile([HID, 1], f32, tag="encb")
            nc.sync.dma_start(out=encb_t[:], in_=encb[:])
            w4_t = cpool.tile([HID, 4], f32, tag="w4")
            nc.sync.dma_start(out=w4_t[:], in_=w4[:])
            clsw_t = cpool.tile([HID, 2], f32, tag="clsw")
            nc.sync.dma_start(out=clsw_t[:], in_=clsw[:])
            clsb_t = cpool.tile([1, 2], f32, tag="clsb")
            nc.sync.dma_start(out=clsb_t[:], in_=clsb[:])
            ident_f = cpool.tile([128, 128], f32, tag="identf")
            nc.sync.dma_start(out=ident_f[:], in_=ident_in[:])
            ident_b = cpool.tile([128, 128], bf16, tag="identb")
            nc.vector.tensor_copy(out=ident_b[:], in_=ident_f[:])
            ones_f = cpool.tile([1, 128], f32, tag="onesf")
            nc.vector.memset(ones_f[:], 1.0)
            inv_all = cpool.tile([128, tpc], f32, tag="invall")
            nc.sync.dma_start(out=inv_all[:], in_=invdeg[:])
            idxA_t = cpool.tile([128, max(tot_a, 1) * 8], i16, tag="idxAt")
            nc.sync.dma_start(out=idxA_t[:], in_=idxA[:])
            idxB_t = cpool.tile([128, max(tot_b, 1) * 8], i16, tag="idxBt")
            nc.sync.dma_start(out=idxB_t[:], in_=idxB[:])
            poison_t = cpool.tile([1, 1], bf16, tag="poison")
            nc.vector.memset(poison_t[:], -10000.0)
            pi_all = [
                cpool.tile([128, tpc], f32, tag=f"piall{l}", name=f"piall{l}")
                for l in (1, 2)
            ]

            def p_phase_and_store(hT_sb, t, layer):
                """hT (f32 [hid, nodes]) -> slab rows [h|pj'] + pi' column."""
                co = t * 128
                lw = 2 * (layer - 1)
                h_ps = ps_tr.tile([128, 128], f32, tag="tr")
                nc.tensor.transpose(out=h_ps[:], in_=hT_sb[:], identity=ident_f[:])
                h_sb = pool.tile([128, 128], bf16, tag="hsb")
                nc.vector.tensor_copy(out=h_sb[:], in_=h_ps[:])
                nc.sync.dma_start(
                    out=slabs[layer - 1][co : co + 128, 0:HID], in_=h_sb[:]
                )
                p_ps = ps_sm.tile([128, 2], f32, tag="sm")
                nc.tensor.matmul(
                    out=p_ps[:], lhsT=hT_sb[:], rhs=w4_t[:, lw : lw + 2],
                    start=True, stop=True,
                )
                # pi' = pi + att_b kept in SBUF for the sigmoid bias
                nc.scalar.add(
                    out=pi_all[layer - 1][:, t : t + 1],
                    in_=p_ps[:, 0:1],
                    add=float(att_bs[layer - 1]),
                )
                pj_ext = pool.tile([128, ROW_W - HID], bf16, tag="pjext")
                nc.vector.memset(pj_ext[:], 0.0)
                nc.vector.tensor_copy(out=pj_ext[:, 0:1], in_=p_ps[:, 1:2])
                nc.sync.dma_start(
                    out=slabs[layer - 1][co : co + 128, HID:ROW_W], in_=pj_ext[:]
                )

            # ---- encoder: hT = relu(encw.T @ xT + encb), then p1 ----
            for t in range(tpc):
                co = t * 128
                xt = [pool.tile([128, 128], f32, tag=f"xt{i}", name=f"xt{i}") for i in range(nk)]
                for i, xx in enumerate(xt):
                    nc.sync.dma_start(
                        out=xx[:], in_=xT[i * 128 : (i + 1) * 128, co : co + 128]
                    )
                hT_ps = ps_tr.tile([128, 128], f32, tag="tr")
                for i in range(nk):
                    nc.tensor.matmul(
                        out=hT_ps[:], lhsT=encw_t[i][:], rhs=xt[i][:],
                        start=(i == 0), stop=(i == nk - 1),
                    )
                hT_sb = pool.tile([128, 128], f32, tag="hTsb")
                nc.scalar.activation(
                    out=hT_sb[:], in_=hT_ps[:], func=AF.Relu, bias=encb_t[:]
                )
                p_phase_and_store(hT_sb, t, layer=1)

            # ---- two message-passing layers ----
            gather_ctr = [0]
            for layer in (1, 2):
                table = tables[layer - 1]
                # poison the hole row's pj' so padding slots get alpha = 0
                nc.sync.dma_start(
                    out=slabs[layer - 1][p["npc"] : p["npc"] + 1, HID : HID + 1],
                    in_=poison_t[:],
                )
                nc.gpsimd.collective_compute(
                    "AllGather",
                    mybir.AluOpType.bypass,
                    replica_groups=[list(range(N_CORES))],
                    ins=[slabs[layer - 1][:]],
                    outs=[table[:]],
                )
                for i in range(tpc):
                    t = proc[i]
                    co = t * 128
                    ca, cb = int(CA[i]), int(CB[i])
                    ct = ca + cb
                    # uniform gather segments of <= SEG chunks, each segment
                    # gathered / sigmoided / weighted / accumulated on its own
                    # so its buffer recycles immediately
                    segs = []
                    for c0 in range(0, ca, SEG):
                        segs.append((0, c0, min(SEG, ca - c0), c0))
                    for c0 in range(0, cb, SEG):
                        segs.append((1, c0, min(SEG, cb - c0), ca + c0))
                    alpha = pool.tile([128, ct], bf16, tag="alpha")
                    acc = ps_acc.tile([128, HID], f32, tag="acc")
                    pi_col = pi_all[layer - 1][:, t : t + 1]
                    for si, (reg, c0, cw, aoff) in enumerate(segs):
                        idx_t, offs, base, rows = (
                            (idxA_t, offs_a, 0, 32768)
                            if reg == 0
                            else (idxB_t, offs_b, gtab - 32768, 32768)
                        )
                        gt = gpool.tile([128, SEG, ROW_W], bf16, tag="g", name="g")
                        nc.gpsimd.dma_gather(
                            out_ap=gt[:, 0:cw, :],
                            in_ap=table[base : base + rows, :],
                            idxs_ap=idx_t[
                                :, (offs[i] + c0) * 8 : (offs[i] + c0 + cw) * 8
                            ],
                            num_idxs=cw * 128,
                            num_idxs_reg=cw * 128,
                            elem_size=ROW_W,
                            single_packet=False,
                            queue_num=gather_ctr[0] % 4,
                        )
                        gather_ctr[0] += 1
                        # alpha = sigmoid(pj + pi') * mask for this segment
                        nc.scalar.activation(
                            out=alpha[:, aoff : aoff + cw, None],
                            in_=gt[:, 0:cw, HID : HID + 1],
                            func=AF.Sigmoid,
                            bias=pi_col,
                        )
                        # msg = alpha (*) rows; slot-sum via identity-matmul
                        # PSUM accumulation (acc[pos] += msg[pos, k, :])
                        msg = dpool.tile([128, SEG, 128], bf16, tag="msg")
                        nc.vector.tensor_tensor(
                            out=msg[:, 0:cw, :],
                            in0=gt[:, 0:cw, 0:HID],
                            in1=alpha[:, aoff : aoff + cw, None].to_broadcast(
                                [128, cw, 128]
                            ),
                            op=mybir.AluOpType.mult,
                        )
                        for k in range(cw):
                            nc.tensor.matmul(
                                out=acc[:],
                                lhsT=ident_b[:],
                                rhs=msg[:, k, :],
                                start=(si == 0 and k == 0),
                                stop=(si == len(segs) - 1 and k == cw - 1),
                            )
                    inv_col = inv_all[:, t : t + 1]
                        if layer == 1:
                            # h2 = relu(acc * invdeg); p2 phase + stores
                            h2_sb = pool.tile([128, 128], f32, tag="h2sb")
                            nc.scalar.activation(
                                out=h2_sb[:], in_=acc[:], func=AF.Relu, scale=inv_col
                            )
                            hT2_ps = ps_tr.tile([128, 128], f32, tag="tr")
                            nc.tensor.transpose(
                                out=hT2_ps[:], in_=h2_sb[:], identity=ident_f[:]
                            )
                            hT2_sb = pool.tile([128, 128], f32, tag="hT2sb")
                            nc.vector.tensor_copy(out=hT2_sb[:], in_=hT2_ps[:])
                            p_phase_and_store(hT2_sb, t, layer=2)
                        else:
                            # logits = (acc * invdeg) @ clsw + clsb
                            m_sb = pool.tile([128, 128], f32, tag="msb")
                            nc.scalar.mul(out=m_sb[:], in_=acc[:], mul=inv_col)
                            mT_ps = ps_tr.tile([128, 128], f32, tag="tr")
                            nc.tensor.transpose(
                                out=mT_ps[:], in_=m_sb[:], identity=ident_f[:]
                            )
                            mT_sb = pool.tile([128, 128], f32, tag="mTsb")
                            nc.vector.tensor_copy(out=mT_sb[:], in_=mT_ps[:])
                            lg_ps = ps_sm.tile([128, 2], f32, tag="sm")
                            nc.tensor.matmul(
                                out=lg_ps[:], lhsT=mT_sb[:], rhs=clsw_t[:],
                                start=True, stop=False,
                            )
                            nc.tensor.matmul(
                                out=lg_ps[:], lhsT=ones_f[:], rhs=clsb_t[:],
                                start=False, stop=True,
                            )
                            lg_sb = pool.tile([128, 2], f32, tag="lgsb")
                            nc.vector.tensor_copy(out=lg_sb[:], in_=lg_ps[:])
                            nc.sync.dma_start(
                                out=logits[co : co + 128, :], in_=lg_sb[:]
                            )

    nc.compile()
    return nc


_CACHE = {}


def kernel(**inputs):
    _install_axon_ntff_hook()
    from concourse import bass_utils

    bass_utils.upload_artifacts = lambda tmpdir: tmpdir

    x = np.asarray(inputs["x"], dtype=np.float32)
    edge_index = np.asarray(inputs["edge_index"])
    p = _host_prep(x, edge_index)

    consts = dict(
        att1_b=float(np.asarray(inputs["att1_b"]).reshape(-1)[0]),
        att2_b=float(np.asarray(inputs["att2_b"]).reshape(-1)[0]),
    )
    key = (tuple(p["CA"]), tuple(p["CB"]), consts["att1_b"], consts["att2_b"])
    if key not in _CACHE:
        _CACHE[key] = _build_program(p, consts)
    nc = _CACHE[key]

    w4 = np.concatenate(
        [
            np.asarray(inputs["att1_w"], dtype=np.float32).reshape(2, HID).T,
            np.asarray(inputs["att2_w"], dtype=np.float32).reshape(2, HID).T,
        ],
        axis=1,
    )  # [HID, 4] = [wi1, wj1, wi2, wj2]
    common = dict(
        encw=np.ascontiguousarray(np.asarray(inputs["enc_w"], dtype=np.float32)),
        encb=np.asarray(inputs["enc_b"], dtype=np.float32).reshape(HID, 1),
        w4=np.ascontiguousarray(w4),
        clsw=np.ascontiguousarray(np.asarray(inputs["cls_w"], dtype=np.float32)),
        clsb=np.asarray(inputs["cls_b"], dtype=np.float32).reshape(1, 2),
        ident=np.eye(128, dtype=np.float32),
    )
    in_maps = []
    for c in range(N_CORES):
        in_maps.append(
            dict(
                xT=np.ascontiguousarray(p["xT"][c]),
                idxA=np.ascontiguousarray(p["idx_a16"][c]),
                idxB=np.ascontiguousarray(p["idx_b16"][c]),
                invdeg=np.ascontiguousarray(p["invdeg"][c]),
                **common,
            )
        )

    res = bass_utils.run_bass_kernel_spmd(nc, in_maps, core_ids=list(range(N_CORES)))
    kernel.last_result = res

    N = p["N"]
    out = np.zeros((N, 2), dtype=np.float32)
    for c in range(N_CORES):
        m = p["owner"] == c
        out[m] = np.asarray(res.results[c]["logits"], dtype=np.float32)[
            p["slabrow"][m]
        ]
    return out

